# revision 13
# baseline (speedup 1.0000x reference)
"""ASTGCN block Trainium2 kernel (v2).

Strategy: 8 cores; core c handles batch b = c//2, time-half h = c%2 (8 output
timesteps each, data-parallel over B and T). Attention (temporal Et, spatial
S) is per-b and replicated on the 2 cores sharing a b. The sparse graph
propagation is reformulated as dense (N,N) matmuls: the edge-scatter of the
symmetric norm is accumulated host-side into a dense W (the +I/-I self-loop
terms cancel), so  prop1(h) = (W*S) @ h  and  prop2(h) = W @ h.

v2 changes vs baseline:
- Input DMAs ordered by first use (Pb/Pf/Xn first) and X tensors split in
  halves so attention matmuls start ~5us in instead of after all loads.
- Single activation-table regime: sigmoid via tanh (0.5*tanh(x/2)+0.5, in
  the exp table) and LN rstd via exp(-0.5*ln(var+eps)); only one table
  switch in the whole program (exp_and_others -> natural_log_exp...).
- LayerNorm runs in pair layout: per-pair stats via ones-block matmuls on
  PE (reduce over the f partition rows), rstd/-mu*rstd broadcast back with
  block matmuls; no transposes of the conv output at all.
- Output stored in pair layout as bf16; host does the final (f,n) -> (n,f)
  transpose and fp32 upcast.
- cheb -> conv -> LN -> store software-pipelined across the 5 timestep
  pairs to keep PE dense (p-state) and overlap store DMAs with compute.

Per-core time axis is PERMUTED so the program is identical SPMD: slot t' maps
to global t via tmap (identity for h=0, rotated by 6 for h=1); all
t-dependent weights (be, Ve, Ws1, UW) are permuted host-side to match.
"""

import numpy as np

B, N, F, T = 4, 512, 64, 16
P = 128
CH = N // P            # 4 n-chunks
NSLOT = 10             # cheb window timesteps per core (5 pairs)
NP = NSLOT // 2        # 5 pairs
LN_EPS = 1e-5

PBW = 1191             # packed bf16 constant width
PFW = 148              # packed f32 constant width

_CACHE = {}


def _build_program():
    import sys
    if '/opt/trn_rl_repo' not in sys.path:
        sys.path.insert(0, '/opt/trn_rl_repo')
    from contextlib import ExitStack
    import concourse.bass as bass
    import concourse.tile as tile
    from concourse import bacc, mybir

    dt = mybir.dt
    AL = mybir.AluOpType
    AF = mybir.ActivationFunctionType
    AX = mybir.AxisListType
    f32 = dt.float32
    bf16 = dt.bfloat16

    nc = bacc.Bacc("TRN2", target_bir_lowering=False, debug=False, num_devices=1)

    def din(name, shape, d=bf16):
        return nc.dram_tensor(name, list(shape), d, kind="ExternalInput").ap()

    XnD   = din("Xn", (N, T * F))
    XwD   = din("Xw", (8, P, N))
    UWD   = din("UW", (8, P, 48))
    bshD  = din("bsh", (N, N))          # 0.5 * bs
    VsTD  = din("VsT", (N, N))
    WTD   = din("WT", (N, N))
    WpkD  = din("Wpk", (7, P, P))
    PbD   = din("Pb", (P, PBW))
    PfD   = din("Pf", (P, PFW), f32)
    ZoutD = nc.dram_tensor("Zout", [NP * P, N], bf16, kind="ExternalOutput").ap()

    with tile.TileContext(nc) as tc, ExitStack() as ctx:
        sg = ctx.enter_context(tc.tile_pool(name="sg", bufs=1))
        big = ctx.enter_context(tc.tile_pool(name="big", bufs=5, space="PSUM"))
        sml = ctx.enter_context(tc.tile_pool(name="sml", bufs=2, space="PSUM"))
        hlf = ctx.enter_context(tc.tile_pool(name="hlf", bufs=1, space="PSUM"))
        xhp = ctx.enter_context(tc.tile_pool(name="xhp", bufs=7))
        txp = ctx.enter_context(tc.tile_pool(name="txp", bufs=3))
        lnp = ctx.enter_context(tc.tile_pool(name="lnp", bufs=3))

        # ------------- input DMAs, ordered by first use -------------
        Pb = sg.tile([P, PBW], bf16, tag="pb")
        nc.sync.dma_start(out=Pb[:], in_=PbD)
        Pf = sg.tile([P, PFW], f32, tag="pf")
        nc.sync.dma_start(out=Pf[:], in_=PfD)
        XnA = sg.tile([P, 2, T * F], bf16, tag="xna")
        XnB = sg.tile([P, 2, T * F], bf16, tag="xnb")
        XnDr = XnD.rearrange("(k p) t -> p k t", k=CH)
        nc.sync.dma_start(out=XnA[:], in_=XnDr[:, 0:2, :])
        nc.sync.dma_start(out=XnB[:], in_=XnDr[:, 2:4, :])
        UWAll = sg.tile([P, 8, 48], bf16, tag="uwall")
        nc.sync.dma_start(out=UWAll[:], in_=UWD.rearrange("s p n -> p s n"))
        XwA = sg.tile([P, 4, N], bf16, tag="xwa")
        XwB = sg.tile([P, 4, N], bf16, tag="xwb")
        XwDr = XwD.rearrange("s p n -> p s n")
        nc.sync.dma_start(out=XwA[:], in_=XwDr[:, 0:4, :])
        nc.sync.dma_start(out=XwB[:], in_=XwDr[:, 4:8, :])
        bsAll = sg.tile([P, CH, N], bf16, tag="bsall")
        nc.sync.dma_start(out=bsAll[:], in_=bshD.rearrange("(k p) n -> p k n", k=CH))
        VsTAll = sg.tile([P, CH, N], bf16, tag="vstall")
        nc.sync.dma_start(out=VsTAll[:], in_=VsTD.rearrange("(k p) n -> p k n", k=CH))
        WTAll = sg.tile([P, CH, N], bf16, tag="wtall")
        nc.sync.dma_start(out=WTAll[:], in_=WTD.rearrange("(k p) n -> p k n", k=CH))
        Wpk = sg.tile([P, 7, P], bf16, tag="wpk")
        nc.sync.dma_start(out=Wpk[:], in_=WpkD.rearrange("w p c -> p w c"))

        Xn = [XnA[:, 0, :], XnA[:, 1, :], XnB[:, 0, :], XnB[:, 1, :]]
        Xw = [XwA[:, s, :] for s in range(4)] + [XwB[:, s, :] for s in range(4)]
        UW = [UWAll[:, s, :] for s in range(8)]
        bsh = [bsAll[:, k, :] for k in range(CH)]
        VsT = [VsTAll[:, k, :] for k in range(CH)]
        WT = [WTAll[:, k, :] for k in range(CH)]
        WcP = [Wpk[:, k, :] for k in range(3)]
        Lprev, Lmid, Lnext, WrP = (Wpk[:, 3, :], Wpk[:, 4, :], Wpk[:, 5, :],
                                   Wpk[:, 6, :])
        # packed bf16 layout
        U1r = Pb[:, 0:4]
        Ws2d = Pb[:, 4:20]
        VeT = Pb[0:16, 20:36]
        Ws1 = Pb[0:16, 36:37]
        ones1 = Pb[0:1, 37:165]
        I128b = Pb[:, 165:293]
        U2 = Pb[0:64, 293:805]
        I16r = Pb[0:1, 805:1061]     # I16 rows flattened: e_t = [0:1, 16t:16t+16]
        B2 = Pb[:, 1061:1063]        # (128,2) block col-indicator * 1/64
        B2T = Pb[0:2, 1063:1191]     # (2,128) block row-indicator
        # packed f32 layout
        gamP = Pf[:, 0:1]
        nbetP = Pf[:, 1:2]           # -beta
        bch = Pf[:, 2:3]
        btr = Pf[:, 3:4]
        I128f = Pf[:, 4:132]
        bePh = Pf[0:16, 132:148]     # 0.5 * be (permuted)

        zerot = sg.tile([P, N], bf16, tag="zerot")
        nc.vector.memset(zerot[:], 0.0)
        epsP = sg.tile([P, 1], f32, tag="epsP")
        nc.vector.memset(epsP[:], LN_EPS)

        # persistent sbuf intermediates
        G = [sg.tile([P, N], bf16, tag=f"g{k}", name=f"g{k}") for k in range(CH)]
        Ex = [sg.tile([P, N], bf16, tag=f"ex{k}", name=f"ex{k}") for k in range(CH)]
        A1T = [sg.tile([P, N], bf16, tag=f"a1t{k}", name=f"a1t{k}") for k in range(CH)]
        dSv = [sg.tile([P, 1], f32, tag=f"dsv{k}", name=f"dsv{k}") for k in range(CH)]
        Tx0n = [sg.tile([P, T * F], bf16, tag=f"tx0n{k}", name=f"tx0n{k}")
                for k in range(CH)]
        dSB = sg.tile([P, N], bf16, tag="dsb")

        # =====================================================
        # Attention phase
        # =====================================================
        # ---- lhs0[(t,f)] = sum_n U1[n] X[n,(t,f)]  -> (1,1024)
        L0a = sml.tile([1, 512], f32, tag="sml", name="l0a")
        for k in range(CH):
            nc.tensor.matmul(L0a[:], U1r[:, k:k + 1], Xn[k][:, 0:512],
                             start=(k == 0), stop=(k == CH - 1))
        L0b = sml.tile([1, 512], f32, tag="sml", name="l0b")
        for k in range(CH):
            nc.tensor.matmul(L0b[:], U1r[:, k:k + 1], Xn[k][:, 512:1024],
                             start=(k == 0), stop=(k == CH - 1))
        lhs0row = sg.tile([1, T * F], bf16, tag="lhs0row")
        nc.vector.tensor_copy(lhs0row[:, 0:512], L0a[:])
        nc.vector.tensor_copy(lhs0row[:, 512:1024], L0b[:])
        # reshape to (64,16) via 16 rank-1 matmuls against identity rows
        l0Fp = sml.tile([F, T], f32, tag="sml", name="l0fp")
        for t in range(T):
            nc.tensor.matmul(l0Fp[:], lhs0row[0:1, 64 * t:64 * t + 64],
                             I16r[0:1, 16 * t:16 * t + 16],
                             start=(t == 0), stop=(t == T - 1))
        lhs0F = sg.tile([F, T], bf16, tag="lhs0f")
        nc.vector.tensor_copy(lhs0F[:], l0Fp[:])

        # ---- lhs2T chunks (n,16) = U2[:,chunk].T @ lhs0F
        lhs2T = []
        for k in range(CH):
            pt = sml.tile([P, T], f32, tag="sml", name="l2t")
            nc.tensor.matmul(pt[:], U2[:, k * P:(k + 1) * P], lhs0F[:],
                             start=True, stop=True)
            st = sg.tile([P, T], bf16, tag=f"l2ts{k}", name=f"l2ts{k}")
            nc.vector.tensor_copy(st[:], pt[:])
            lhs2T.append(st)

        # ---- R48: rows 0:16 rhs3T (Ws3), rows 32:48 rhs_tT (U3)
        R48p = sml.tile([48, N], f32, tag="sml", name="r48")
        for s in range(8):
            nc.tensor.matmul(R48p[:], UW[s][:, :], Xw[s][:, :],
                             start=(s == 0), stop=(s == 7))
        R48 = sg.tile([48, N], bf16, tag="r48s")
        nc.scalar.copy(R48[:], R48p[:])

        # ---- rhs_tn chunks: transpose R48[32:48]
        rhs_tn = []
        for k in range(CH):
            pt = hlf.tile([P, T], bf16, tag="hlf", name="rtn")
            nc.tensor.transpose(pt[:], R48[32:48, k * P:(k + 1) * P],
                                I128b[32:48, 32:48])
            st = sg.tile([P, T], bf16, tag=f"rtns{k}", name=f"rtns{k}")
            nc.vector.tensor_copy(st[:], pt[:])
            rhs_tn.append(st)

        # ---- P0 (16,16) = lhs_t @ rhs_t ; sigmoid via tanh:
        # sig = 0.5*tanh(0.5*(P0+be)) + 0.5
        P0p = sml.tile([T, T], f32, tag="sml", name="p0")
        for k in range(CH):
            nc.tensor.matmul(P0p[:], lhs2T[k][:], rhs_tn[k][:],
                             start=(k == 0), stop=(k == CH - 1))
        sig = sg.tile([T, T], bf16, tag="sig")
        nc.vector.scalar_tensor_tensor(sig[:], P0p[:], 0.5, bePh[:],
                                       op0=AL.mult, op1=AL.add)
        nc.scalar.activation(sig[:], sig[:], AF.Tanh)
        nc.vector.tensor_scalar(sig[:], sig[:], 0.5, 0.5,
                                op0=AL.mult, op1=AL.add)

        # ---- E1^T = sig^T @ Ve^T directly ; softmax over free dim
        E1Tp = sml.tile([T, T], f32, tag="sml", name="e1t")
        nc.tensor.matmul(E1Tp[:], sig[:], VeT[:], start=True, stop=True)
        E1Ts = sg.tile([T, T], bf16, tag="e1ts")
        nc.vector.tensor_copy(E1Ts[:], E1Tp[:])
        # values are O(1e-1): skip the max-subtraction for softmax
        sume = sg.tile([T, 1], f32, tag="sume")
        EtT = sg.tile([T, T], bf16, tag="ett")
        nc.scalar.activation(EtT[:], E1Ts[:], AF.Exp,
                             scale=1.0, accum_out=sume[:, 0:1])
        rse = sg.tile([T, 1], f32, tag="rse")
        nc.vector.reciprocal(rse[:], sume[:])
        nc.vector.tensor_scalar(EtT[:], EtT[:], rse[:, 0:1], None, op0=AL.mult)
        Etp = hlf.tile([T, T], bf16, tag="hlf", name="etp")
        nc.tensor.transpose(Etp[:], EtT[:], I128b[0:16, 0:16])
        Et = sg.tile([T, T], bf16, tag="et")
        nc.vector.tensor_copy(Et[:], Etp[:])

        # ---- w1e row (1,16) = Ws1.T @ EtT ; broadcast to (128,16)
        w1p = sml.tile([1, T], f32, tag="sml", name="w1p")
        nc.tensor.matmul(w1p[:], Ws1[:], EtT[:], start=True, stop=True)
        w1row = sg.tile([1, T], bf16, tag="w1row")
        nc.scalar.copy(w1row[:], w1p[:])
        w1Bp = sml.tile([P, T], f32, tag="sml", name="w1bp")
        nc.tensor.matmul(w1Bp[:], ones1[:], w1row[:], start=True, stop=True)
        w1B = sg.tile([P, T], bf16, tag="w1b")
        nc.vector.tensor_copy(w1B[:], w1Bp[:])

        # ---- w1Bpair[p=(v,f), s] = w1e[2s+v]
        w1Bp2 = sg.tile([P, 8], bf16, tag="w1bp2")
        nc.vector.tensor_copy(w1Bp2[0:64, :], w1B[0:64, 0:T:2])
        nc.vector.tensor_copy(w1Bp2[64:128, :], w1B[64:128, 1:T:2])
        # ---- Ws2wP[p=(v,f), s, t] = Ws2d[p,t] * w1e[2s+v]   (128, 8, 16)
        Ws2w = sg.tile([P, 8, T], bf16, tag="ws2w")
        nc.vector.tensor_tensor(
            Ws2w[:],
            Ws2d[:].unsqueeze(1).broadcast_to((P, 8, T)),
            w1Bp2[:].unsqueeze(2).broadcast_to((P, 8, T)),
            op=AL.mult)

        # ---- lhs_sT (16, 512) = sum_t1 (Ws2*w1e[t1]).T @ X^T[t1]
        lsTp = sml.tile([T, N], f32, tag="sml", name="lst")
        for s in range(8):
            nc.tensor.matmul(lsTp[:], Ws2w[:, s, :], Xw[s][:, :],
                             start=(s == 0), stop=(s == 7))
        lsT = sg.tile([T, N], bf16, tag="lsts")
        nc.scalar.copy(lsT[:], lsTp[:])

        # ---- rhs_s (16, 512) = Et-weighted rhs3
        rsp = sml.tile([T, N], f32, tag="sml", name="rsp")
        nc.tensor.matmul(rsp[:], Et[:], R48[0:16, :], start=True, stop=True)
        rss = sg.tile([T, N], bf16, tag="rss")
        nc.scalar.copy(rss[:], rsp[:])

        # ---- P chunks + G = sigmoid(P + bs) = 0.5*tanh(0.5*P + bsh) + 0.5
        for k in range(CH):
            Pp = big.tile([P, N], f32, tag="big", name="pp")
            nc.tensor.matmul(Pp[:], lsT[:, k * P:(k + 1) * P], rss[:],
                             start=True, stop=True)
            nc.vector.scalar_tensor_tensor(G[k][:], Pp[:], 0.5, bsh[k][:],
                                           op0=AL.mult, op1=AL.add)
            nc.scalar.activation(G[k][:], G[k][:], AF.Tanh)
            nc.vector.tensor_scalar(G[k][:], G[k][:], 0.5, 0.5,
                                    op0=AL.mult, op1=AL.add)

        # ---- M1T chunks (c-part, r) + masked softmax -> A1T, dS
        for c in range(CH):
            Mp = big.tile([P, N], f32, tag="big", name="mp")
            for k in range(CH):
                nc.tensor.matmul(Mp[:], G[k][:, c * P:(c + 1) * P], VsT[k][:],
                                 start=(k == 0), stop=(k == CH - 1))
            sme = sg.tile([P, 1], f32, tag=f"sme{c}", name=f"sme{c}")
            nc.scalar.activation(Ex[c][:], Mp[:], AF.Exp,
                                 scale=1.0, accum_out=sme[:, 0:1])
            rcp = sg.tile([P, 1], f32, tag=f"rcp{c}", name=f"rcp{c}")
            nc.vector.reciprocal(rcp[:], sme[:])
            # A1T = (Ex * rcp) * WT   (= S^T o W^T)
            nc.vector.scalar_tensor_tensor(A1T[c][:], Ex[c][:], rcp[:, 0:1],
                                           WT[c][:], op0=AL.mult, op1=AL.mult)
            # diag: dS = sum_r (Ex*rcp)*I over the diagonal block
            dtmp = sg.tile([P, P], bf16, tag="dtmp")
            nc.vector.scalar_tensor_tensor(dtmp[:], Ex[c][:, c * P:(c + 1) * P],
                                           rcp[:, 0:1], I128b[:],
                                           op0=AL.mult, op1=AL.mult)
            nc.vector.tensor_reduce(dSv[c][:], dtmp[:], axis=AX.X, op=AL.add)

        # ---- dS row + broadcast tile (128, 512)
        dSrp = sml.tile([1, N], f32, tag="sml", name="dsrp")
        for c in range(CH):
            nc.tensor.transpose(dSrp[:, c * P:(c + 1) * P], dSv[c][:], I128f[:])
        dSrow = sg.tile([1, N], bf16, tag="dsrow")
        nc.scalar.copy(dSrow[:], dSrp[:])
        dSBp = sml.tile([P, N], f32, tag="sml", name="dsbp")
        nc.tensor.matmul(dSBp[:], ones1[:], dSrow[:], start=True, stop=True)
        nc.scalar.copy(dSB[:], dSBp[:])

        # ---- Tx0 in n-layout (all t at once)
        for k in range(CH):
            nc.vector.tensor_scalar(Tx0n[k][:], Xn[k][:], dSv[k][:, 0:1], None,
                                    op0=AL.mult)

        # =====================================================
        # Cheb + conv + LN, software-pipelined per pair
        # =====================================================
        Tx0P = {}
        TAp = {}
        Tx1T = {}
        ptA = {}
        Tx1n = {}
        TBp = {}
        Tx2T = {}
        TCp = {}
        XhP = {-1: zerot, NP: zerot}
        TDp = {}
        ZT = {}

        def e_tx0p(q):
            t = txp.tile([P, N], bf16, tag="tx0p", name=f"tx0p{q}")
            nc.vector.tensor_tensor(t[:], Xw[q][:], dSB[:], op=AL.mult)
            Tx0P[q] = t

        def e_ta(q):
            p = big.tile([P, N], f32, tag="big", name=f"ta{q}")
            for k in range(CH):
                lhs = Tx0n[k][:, 2 * q * F:(2 * q + 2) * F]
                nc.tensor.matmul(p[:], lhs, A1T[k][:],
                                 start=(k == 0), stop=(k == CH - 1))
            TAp[q] = p

        def e_b(q):
            t = txp.tile([P, N], bf16, tag="tx1t", name=f"tx1t{q}")
            nc.vector.tensor_copy(t[:], TAp[q][:])
            Tx1T[q] = t

        def e_c(q):
            p = hlf.tile([P, N], bf16, tag="hlf", name=f"pta{q}")
            for k in range(CH):
                nc.tensor.transpose(p[:, k * P:(k + 1) * P],
                                    Tx1T[q][:, k * P:(k + 1) * P], I128b[:])
            ptA[q] = p

        def e_d(q):
            t = txp.tile([P, N], bf16, tag="tx1n", name=f"tx1n{q}")
            nc.scalar.copy(t[:], ptA[q][:])
            Tx1n[q] = t

        def e_e(q):
            p = big.tile([P, N], f32, tag="big", name=f"tb{q}")
            for k in range(CH):
                nc.tensor.matmul(p[:], Tx1n[q][:, k * P:(k + 1) * P], WT[k][:],
                                 start=(k == 0), stop=(k == CH - 1))
            TBp[q] = p

        def e_f(q):
            t = txp.tile([P, N], bf16, tag="tx2t", name=f"tx2t{q}")
            nc.vector.scalar_tensor_tensor(t[:], TBp[q][:], 2.0, Tx0P[q][:],
                                           op0=AL.mult, op1=AL.subtract)
            Tx2T[q] = t

        def e_g(q):
            p = big.tile([P, N], f32, tag="big", name=f"tc{q}")
            nc.tensor.matmul(p[:], WcP[0][:], Tx0P[q][:], start=True, stop=False)
            nc.tensor.matmul(p[:], WcP[1][:], Tx1T[q][:], start=False, stop=False)
            nc.tensor.matmul(p[:], WcP[2][:], Tx2T[q][:], start=False, stop=True)
            TCp[q] = p

        def e_h(q):
            t = xhp.tile([P, N], bf16, tag="xh", name=f"xh{q}")
            nc.scalar.activation(t[:], TCp[q][:], AF.Relu, bias=bch[:, 0:1],
                                 scale=1.0)
            XhP[q] = t

        def e_i(q):
            p = big.tile([P, N], f32, tag="big", name=f"td{q}")
            nc.tensor.matmul(p[:], Lprev[:], XhP[q - 1][:], start=True, stop=False)
            nc.tensor.matmul(p[:], Lmid[:], XhP[q][:], start=False, stop=False)
            nc.tensor.matmul(p[:], Lnext[:], XhP[q + 1][:], start=False, stop=False)
            nc.tensor.matmul(p[:], WrP[:], Xw[q][:], start=False, stop=True)
            TDp[q] = p

        def e_j(q):
            t = lnp.tile([P, N], bf16, tag="zt", name=f"zt{q}")
            nc.scalar.activation(t[:], TDp[q][:], AF.Relu, bias=btr[:, 0:1],
                                 scale=1.0)
            ZT[q] = t

        def e_ln(q):
            z = ZT[q]
            sq = lnp.tile([P, N], bf16, tag="sq", name=f"sq{q}")
            nc.gpsimd.tensor_tensor(sq[:], z[:], z[:], op=AL.mult)
            # stats: rows 0:2 mean per v, rows 32:34 E[x^2] per v (B2 = 1/64)
            # (matmul out base partition must be 0/32/64)
            s12 = sml.tile([34, N], f32, tag="sml", name=f"s12{q}")
            nc.tensor.matmul(s12[0:2, :], B2[:], z[:], start=True, stop=True)
            nc.tensor.matmul(s12[32:34, :], B2[:], sq[:], start=True, stop=True)
            mu = lnp.tile([2, N], f32, tag="mu", name=f"mu{q}")
            nc.scalar.copy(mu[:], s12[0:2, :])
            mu2 = lnp.tile([2, N], f32, tag="mu2", name=f"mu2{q}")
            nc.vector.tensor_tensor(mu2[:], mu[:], mu[:], op=AL.mult)
            var = lnp.tile([2, N], f32, tag="var", name=f"var{q}")
            nc.vector.tensor_tensor(var[:], s12[32:34, :], mu2[:], op=AL.subtract)
            # rstd = exp(-0.5*ln(var+eps))
            rln = lnp.tile([2, N], f32, tag="rln", name=f"rln{q}")
            nc.scalar.activation(rln[:], var[:], AF.Ln, bias=epsP[0:2, 0:1],
                                 scale=1.0)
            rstd = lnp.tile([2, N], bf16, tag="rstd", name=f"rstd{q}")
            nc.scalar.activation(rstd[:], rln[:], AF.Exp, scale=-0.5)
            # nmr = mu * rstd
            nmr = lnp.tile([2, N], bf16, tag="nmr", name=f"nmr{q}")
            nc.vector.tensor_tensor(nmr[:], mu[:], rstd[:], op=AL.mult)
            # broadcast to 128 partitions
            rBp = big.tile([P, N], f32, tag="big", name=f"rbp{q}")
            nc.tensor.matmul(rBp[:], B2T[:], rstd[:], start=True, stop=True)
            nBp = big.tile([P, N], f32, tag="big", name=f"nbp{q}")
            nc.tensor.matmul(nBp[:], B2T[:], nmr[:], start=True, stop=True)
            # w = (z*rB)*gam - (nB*gam - bet)
            u = lnp.tile([P, N], bf16, tag="u", name=f"u{q}")
            nc.vector.tensor_tensor(u[:], z[:], rBp[:], op=AL.mult)
            nB2 = lnp.tile([P, N], bf16, tag="nb2", name=f"nb2{q}")
            nc.scalar.activation(nB2[:], nBp[:], AF.Identity,
                                 bias=nbetP[:, 0:1], scale=gamP[:, 0:1])
            w = lnp.tile([P, N], bf16, tag="w", name=f"w{q}")
            nc.vector.scalar_tensor_tensor(w[:], u[:], gamP[:, 0:1], nB2[:],
                                           op0=AL.mult, op1=AL.subtract)
            nc.sync.dma_start(out=ZoutD[q * P:(q + 1) * P, :], in_=w[:])

        # pipeline drive
        e_tx0p(0)
        e_ta(0)
        e_b(0)
        e_tx0p(1)
        e_ta(1)
        e_c(0)
        e_d(0)
        e_e(0)
        e_f(0)
        e_g(0)
        e_h(0)
        e_b(1)
        e_tx0p(2)
        for q in range(2, NP):
            e_ta(q)
            e_c(q - 1)
            e_d(q - 1)
            e_e(q - 1)
            e_f(q - 1)
            e_g(q - 1)
            e_h(q - 1)
            e_b(q)
            if q + 1 < NP:
                e_tx0p(q + 1)
            e_i(q - 2)
            e_j(q - 2)
            e_ln(q - 2)
        e_c(NP - 1)
        e_d(NP - 1)
        e_e(NP - 1)
        e_f(NP - 1)
        e_g(NP - 1)
        e_h(NP - 1)
        e_i(NP - 2)
        e_j(NP - 2)
        e_ln(NP - 2)
        e_i(NP - 1)
        e_j(NP - 1)
        e_ln(NP - 1)

    nc.compile()
    return nc


def _host_prep(inputs):
    import ml_dtypes
    bf = ml_dtypes.bfloat16

    X = np.asarray(inputs['X'], np.float32)
    edge_index = np.asarray(inputs['edge_index'])
    U1 = np.asarray(inputs['U1'], np.float32)
    U2 = np.asarray(inputs['U2'], np.float32)
    U3 = np.asarray(inputs['U3'], np.float32)
    be = np.asarray(inputs['be'], np.float32)
    Ve = np.asarray(inputs['Ve'], np.float32)
    Ws1 = np.asarray(inputs['Ws1'], np.float32)
    Ws2 = np.asarray(inputs['Ws2'], np.float32)
    Ws3 = np.asarray(inputs['Ws3'], np.float32)
    bs = np.asarray(inputs['bs'], np.float32)
    Vs = np.asarray(inputs['Vs'], np.float32)
    W_cheb = np.asarray(inputs['W_cheb'], np.float32)
    b_cheb = np.asarray(inputs['b_cheb'], np.float32)
    Wt = np.asarray(inputs['Wt'], np.float32)
    bt = np.asarray(inputs['bt'], np.float32)
    Wr = np.asarray(inputs['Wr'], np.float32)
    br = np.asarray(inputs['br'], np.float32)
    gamma = np.asarray(inputs['gamma'], np.float32)
    beta = np.asarray(inputs['beta'], np.float32)

    # dense symmetric-norm matrix (self-loop +I/-I terms cancel)
    row, col = edge_index[0].astype(np.int64), edge_index[1].astype(np.int64)
    deg = np.zeros(N, np.float32)
    np.add.at(deg, row, 1.0)
    dis = np.where(deg > 0, 1.0 / np.sqrt(np.maximum(deg, 1.0)), 0.0).astype(np.float32)
    wn = -dis[row] * dis[col]
    W = np.zeros((N, N), np.float32)
    np.add.at(W, (row, col), wn)

    # conv block matrices: L[(v,fi),(u,fo)] = Wt[fo,fi,0,dt]
    WtT = [np.ascontiguousarray(Wt[:, :, 0, d].T) for d in range(3)]  # (fi,fo)
    Z64 = np.zeros((F, F), np.float32)
    Lmid = np.block([[WtT[1], WtT[0]], [WtT[2], WtT[1]]]).astype(bf)
    Lprev = np.block([[Z64, Z64], [WtT[0], Z64]]).astype(bf)
    Lnext = np.block([[Z64, WtT[2]], [Z64, Z64]]).astype(bf)
    WrT = np.ascontiguousarray(Wr[:, :, 0, 0].T)
    WrP = np.block([[WrT, Z64], [Z64, WrT]]).astype(bf)
    WcP = np.stack([np.block([[W_cheb[k], Z64], [Z64, W_cheb[k]]]) for k in range(3)]
                   ).astype(bf)

    Wpk = np.stack([WcP[0], WcP[1], WcP[2], Lprev, Lmid, Lnext, WrP])

    Pf = np.zeros((P, PFW), np.float32)
    Pf[:, 0] = np.tile(gamma, 2)
    Pf[:, 1] = np.tile(-beta, 2)
    Pf[:, 2] = np.tile(b_cheb, 2)
    Pf[:, 3] = np.tile(bt + br, 2)
    Pf[:, 4:132] = np.eye(P, dtype=np.float32)

    shared = {
        'bsh': (0.5 * bs[0]).astype(bf),
        'VsT': np.ascontiguousarray(Vs.T).astype(bf),
        'WT': np.ascontiguousarray(W.T).astype(bf),
        'Wpk': Wpk,
    }

    in_maps = []
    for core in range(8):
        b, h = core // 2, core % 2
        tmap = list(range(16)) if h == 0 else list(range(6, 16)) + list(range(6))
        Xp = X[b][:, :, tmap]                              # (N, F, 16)
        Xn = np.ascontiguousarray(Xp.transpose(0, 2, 1).reshape(N, T * F)).astype(bf)
        Xw = np.ascontiguousarray(Xp.transpose(2, 1, 0).reshape(8, P, N)).astype(bf)
        UW = np.zeros((8, P, 48), np.float32)
        for tp in range(16):
            s, v = tp // 2, tp % 2
            UW[s, 64 * v:64 * v + 64, tp] = Ws3
            UW[s, 64 * v:64 * v + 64, 32 + tp] = U3
        Pb = np.zeros((P, PBW), np.float32)
        Pb[:, 0:4] = U1.reshape(4, P).T
        Pb[:, 4:20] = np.vstack([Ws2, Ws2])
        Pb[0:16, 20:36] = Ve[np.ix_(tmap, tmap)].T
        Pb[0:16, 36] = Ws1[tmap]
        Pb[0, 37:165] = 1.0
        Pb[:, 165:293] = np.eye(P, dtype=np.float32)
        Pb[0:64, 293:805] = U2
        Pb[0, 805:1061] = np.eye(T, dtype=np.float32).reshape(-1)
        # B2: (128, 2) block indicator * 1/64 for per-v mean over f
        Pb[0:64, 1061] = 1.0 / 64
        Pb[64:128, 1062] = 1.0 / 64
        # B2T: (2, 128) block indicator for broadcast back
        Pb[0, 1063:1127] = 1.0
        Pb[1, 1127:1191] = 1.0
        Pfc = Pf.copy()
        Pfc[0:16, 132:148] = 0.5 * be[0][np.ix_(tmap, tmap)]
        m = dict(shared)
        m.update({
            'Xn': Xn, 'Xw': Xw, 'UW': UW.astype(bf),
            'Pb': Pb.astype(bf), 'Pf': Pfc,
        })
        in_maps.append(m)
    return in_maps


def kernel(**inputs):
    import sys
    if '/opt/trn_rl_repo' not in sys.path:
        sys.path.insert(0, '/opt/trn_rl_repo')
    from concourse.bass_utils import run_bass_kernel_spmd

    if 'nc' not in _CACHE:
        _CACHE['nc'] = _build_program()
    nc = _CACHE['nc']

    in_maps = _host_prep(inputs)
    res = run_bass_kernel_spmd(nc, in_maps, list(range(8)))
    out = np.zeros((B, N, F, T), np.float32)
    for core in range(8):
        b, h = core // 2, core % 2
        Z = np.asarray(res.results[core]['Zout']).astype(np.float32)
        # rows q*128 + v*64 + f, cols n  ->  (n, f, slot=2q+v)
        Zs = Z.reshape(NP, 2, F, N).transpose(3, 2, 0, 1).reshape(N, F, NSLOT)
        wstart = 0 if h == 0 else 6
        jlo = 0 if h == 0 else 2
        out[b, :, :, wstart + jlo:wstart + jlo + 8] = Zs[:, :, jlo:jlo + 8]
    return out


# revision 16
# speedup vs baseline: 1.0650x; 1.0650x over previous
"""ASTGCN block Trainium2 kernel (v2).

Strategy: 8 cores; core c handles batch b = c//2, time-half h = c%2 (8 output
timesteps each, data-parallel over B and T). Attention (temporal Et, spatial
S) is per-b and replicated on the 2 cores sharing a b. The sparse graph
propagation is reformulated as dense (N,N) matmuls: the edge-scatter of the
symmetric norm is accumulated host-side into a dense W (the +I/-I self-loop
terms cancel), so  prop1(h) = (W*S) @ h  and  prop2(h) = W @ h.

v2 changes vs baseline:
- Input DMAs ordered by first use (Pb/Pf/Xn first) and X tensors split in
  halves so attention matmuls start ~5us in instead of after all loads.
- Single activation-table regime: sigmoid via tanh (0.5*tanh(x/2)+0.5, in
  the exp table) and LN rstd via exp(-0.5*ln(var+eps)); only one table
  switch in the whole program (exp_and_others -> natural_log_exp...).
- LayerNorm runs in pair layout: per-pair stats via ones-block matmuls on
  PE (reduce over the f partition rows), rstd/-mu*rstd broadcast back with
  block matmuls; no transposes of the conv output at all.
- Output stored in pair layout as bf16; host does the final (f,n) -> (n,f)
  transpose and fp32 upcast.
- cheb -> conv -> LN -> store software-pipelined across the 5 timestep
  pairs to keep PE dense (p-state) and overlap store DMAs with compute.

Per-core time axis is PERMUTED so the program is identical SPMD: slot t' maps
to global t via tmap (identity for h=0, rotated by 6 for h=1); all
t-dependent weights (be, Ve, Ws1, UW) are permuted host-side to match.
"""

import numpy as np

B, N, F, T = 4, 512, 64, 16
P = 128
CH = N // P            # 4 n-chunks
NSLOT = 10             # cheb window timesteps per core (5 pairs)
NP = NSLOT // 2        # 5 pairs
LN_EPS = 1e-5

PBW = 1191             # packed bf16 constant width
PFW = 148              # packed f32 constant width

_CACHE = {}


def _build_program():
    import sys
    if '/opt/trn_rl_repo' not in sys.path:
        sys.path.insert(0, '/opt/trn_rl_repo')
    from contextlib import ExitStack
    import concourse.bass as bass
    import concourse.tile as tile
    from concourse import bacc, mybir

    dt = mybir.dt
    AL = mybir.AluOpType
    AF = mybir.ActivationFunctionType
    AX = mybir.AxisListType
    f32 = dt.float32
    bf16 = dt.bfloat16

    nc = bacc.Bacc("TRN2", target_bir_lowering=False, debug=False, num_devices=1)

    def din(name, shape, d=bf16):
        return nc.dram_tensor(name, list(shape), d, kind="ExternalInput").ap()

    XnD   = din("Xn", (N, T * F))
    XwD   = din("Xw", (8, P, N))
    UWD   = din("UW", (8, P, 48))
    bshD  = din("bsh", (N, N))          # 0.5 * bs
    VsTD  = din("VsT", (N, N))
    WTD   = din("WT", (N, N))
    WpkD  = din("Wpk", (7, P, P))
    PbD   = din("Pb", (P, PBW))
    PfD   = din("Pf", (P, PFW), f32)
    ZoutD = nc.dram_tensor("Zout", [NP * P, N], bf16, kind="ExternalOutput").ap()

    with tile.TileContext(nc) as tc, ExitStack() as ctx:
        sg = ctx.enter_context(tc.tile_pool(name="sg", bufs=1))
        big = ctx.enter_context(tc.tile_pool(name="big", bufs=5, space="PSUM"))
        sml = ctx.enter_context(tc.tile_pool(name="sml", bufs=2, space="PSUM"))
        hlf = ctx.enter_context(tc.tile_pool(name="hlf", bufs=1, space="PSUM"))
        xhp = ctx.enter_context(tc.tile_pool(name="xhp", bufs=7))
        txp = ctx.enter_context(tc.tile_pool(name="txp", bufs=3))
        lnp = ctx.enter_context(tc.tile_pool(name="lnp", bufs=3))

        # ------------- input DMAs, ordered by first use -------------
        Pb = sg.tile([P, PBW], bf16, tag="pb")
        nc.sync.dma_start(out=Pb[:], in_=PbD)
        Pf = sg.tile([P, PFW], f32, tag="pf")
        nc.sync.dma_start(out=Pf[:], in_=PfD)
        XnA = sg.tile([P, 2, T * F], bf16, tag="xna")
        XnB = sg.tile([P, 2, T * F], bf16, tag="xnb")
        XnDr = XnD.rearrange("(k p) t -> p k t", k=CH)
        nc.sync.dma_start(out=XnA[:], in_=XnDr[:, 0:2, :])
        nc.sync.dma_start(out=XnB[:], in_=XnDr[:, 2:4, :])
        UWAll = sg.tile([P, 8, 48], bf16, tag="uwall")
        nc.sync.dma_start(out=UWAll[:], in_=UWD.rearrange("s p n -> p s n"))
        XwA = sg.tile([P, 4, N], bf16, tag="xwa")
        XwB = sg.tile([P, 4, N], bf16, tag="xwb")
        XwDr = XwD.rearrange("s p n -> p s n")
        nc.sync.dma_start(out=XwA[:], in_=XwDr[:, 0:4, :])
        nc.sync.dma_start(out=XwB[:], in_=XwDr[:, 4:8, :])
        bsAll = sg.tile([P, CH, N], bf16, tag="bsall")
        nc.sync.dma_start(out=bsAll[:], in_=bshD.rearrange("(k p) n -> p k n", k=CH))
        VsTAll = sg.tile([P, CH, N], bf16, tag="vstall")
        nc.sync.dma_start(out=VsTAll[:], in_=VsTD.rearrange("(k p) n -> p k n", k=CH))
        WTAll = sg.tile([P, CH, N], bf16, tag="wtall")
        nc.sync.dma_start(out=WTAll[:], in_=WTD.rearrange("(k p) n -> p k n", k=CH))
        Wpk = sg.tile([P, 7, P], bf16, tag="wpk")
        nc.sync.dma_start(out=Wpk[:], in_=WpkD.rearrange("w p c -> p w c"))

        Xn = [XnA[:, 0, :], XnA[:, 1, :], XnB[:, 0, :], XnB[:, 1, :]]
        Xw = [XwA[:, s, :] for s in range(4)] + [XwB[:, s, :] for s in range(4)]
        UW = [UWAll[:, s, :] for s in range(8)]
        bsh = [bsAll[:, k, :] for k in range(CH)]
        VsT = [VsTAll[:, k, :] for k in range(CH)]
        WT = [WTAll[:, k, :] for k in range(CH)]
        WcP = [Wpk[:, k, :] for k in range(3)]
        Lprev, Lmid, Lnext, WrP = (Wpk[:, 3, :], Wpk[:, 4, :], Wpk[:, 5, :],
                                   Wpk[:, 6, :])
        # packed bf16 layout
        U1r = Pb[:, 0:4]
        Ws2d = Pb[:, 4:20]
        VeT = Pb[0:16, 20:36]
        Ws1 = Pb[0:16, 36:37]
        ones1 = Pb[0:1, 37:165]
        I128b = Pb[:, 165:293]
        U2 = Pb[0:64, 293:805]
        I16r = Pb[0:1, 805:1061]     # I16 rows flattened: e_t = [0:1, 16t:16t+16]
        B2 = Pb[:, 1061:1063]        # (128,2) block col-indicator * 1/64
        B2T = Pb[0:2, 1063:1191]     # (2,128) block row-indicator
        # packed f32 layout
        gamP = Pf[:, 0:1]
        nbetP = Pf[:, 1:2]           # -beta
        bch = Pf[:, 2:3]
        btr = Pf[:, 3:4]
        I128f = Pf[:, 4:132]
        bePh = Pf[0:16, 132:148]     # 0.5 * be (permuted)

        zerot = sg.tile([P, N], bf16, tag="zerot")
        nc.vector.memset(zerot[:], 0.0)
        epsP = sg.tile([P, 1], f32, tag="epsP")
        nc.vector.memset(epsP[:], LN_EPS)

        # persistent sbuf intermediates
        G = [sg.tile([P, N], bf16, tag=f"g{k}", name=f"g{k}") for k in range(CH)]
        Ex = [sg.tile([P, N], bf16, tag=f"ex{k}", name=f"ex{k}") for k in range(CH)]
        A1T = [sg.tile([P, N], bf16, tag=f"a1t{k}", name=f"a1t{k}") for k in range(CH)]
        dSv = [sg.tile([P, 1], f32, tag=f"dsv{k}", name=f"dsv{k}") for k in range(CH)]
        Tx0n = [sg.tile([P, T * F], bf16, tag=f"tx0n{k}", name=f"tx0n{k}")
                for k in range(CH)]
        dSB = sg.tile([P, N], bf16, tag="dsb")

        # =====================================================
        # Attention phase
        # =====================================================
        # ---- lhs0[(t,f)] = sum_n U1[n] X[n,(t,f)]  -> (1,1024)
        L0a = sml.tile([1, 512], f32, tag="sml", name="l0a")
        for k in range(CH):
            nc.tensor.matmul(L0a[:], U1r[:, k:k + 1], Xn[k][:, 0:512],
                             start=(k == 0), stop=(k == CH - 1))
        L0b = sml.tile([1, 512], f32, tag="sml", name="l0b")
        for k in range(CH):
            nc.tensor.matmul(L0b[:], U1r[:, k:k + 1], Xn[k][:, 512:1024],
                             start=(k == 0), stop=(k == CH - 1))
        lhs0row = sg.tile([1, T * F], bf16, tag="lhs0row")
        nc.vector.tensor_copy(lhs0row[:, 0:512], L0a[:])
        nc.vector.tensor_copy(lhs0row[:, 512:1024], L0b[:])
        # reshape to (64,16) via 16 rank-1 matmuls against identity rows
        l0Fp = sml.tile([F, T], f32, tag="sml", name="l0fp")
        for t in range(T):
            nc.tensor.matmul(l0Fp[:], lhs0row[0:1, 64 * t:64 * t + 64],
                             I16r[0:1, 16 * t:16 * t + 16],
                             start=(t == 0), stop=(t == T - 1))
        lhs0F = sg.tile([F, T], bf16, tag="lhs0f")
        nc.vector.tensor_copy(lhs0F[:], l0Fp[:])

        # ---- lhs2T chunks (n,16) = U2[:,chunk].T @ lhs0F
        lhs2T = []
        for k in range(CH):
            pt = sml.tile([P, T], f32, tag="sml", name="l2t")
            nc.tensor.matmul(pt[:], U2[:, k * P:(k + 1) * P], lhs0F[:],
                             start=True, stop=True)
            st = sg.tile([P, T], bf16, tag=f"l2ts{k}", name=f"l2ts{k}")
            nc.vector.tensor_copy(st[:], pt[:])
            lhs2T.append(st)

        # ---- R48: rows 0:16 rhs3T (Ws3), rows 32:48 rhs_tT (U3)
        R48p = sml.tile([48, N], f32, tag="sml", name="r48")
        for s in range(8):
            nc.tensor.matmul(R48p[:], UW[s][:, :], Xw[s][:, :],
                             start=(s == 0), stop=(s == 7))
        R48 = sg.tile([48, N], bf16, tag="r48s")
        nc.scalar.copy(R48[:], R48p[:])

        # ---- rhs_tn chunks: transpose R48[32:48]
        rhs_tn = []
        for k in range(CH):
            pt = hlf.tile([P, T], bf16, tag="hlf", name="rtn")
            nc.tensor.transpose(pt[:], R48[32:48, k * P:(k + 1) * P],
                                I128b[32:48, 32:48])
            st = sg.tile([P, T], bf16, tag=f"rtns{k}", name=f"rtns{k}")
            nc.vector.tensor_copy(st[:], pt[:])
            rhs_tn.append(st)

        # ---- P0 (16,16) = lhs_t @ rhs_t ; sigmoid via tanh:
        # sig = 0.5*tanh(0.5*(P0+be)) + 0.5
        P0p = sml.tile([T, T], f32, tag="sml", name="p0")
        for k in range(CH):
            nc.tensor.matmul(P0p[:], lhs2T[k][:], rhs_tn[k][:],
                             start=(k == 0), stop=(k == CH - 1))
        sig = sg.tile([T, T], bf16, tag="sig")
        nc.vector.scalar_tensor_tensor(sig[:], P0p[:], 0.5, bePh[:],
                                       op0=AL.mult, op1=AL.add)
        nc.scalar.activation(sig[:], sig[:], AF.Tanh)
        nc.vector.tensor_scalar(sig[:], sig[:], 0.5, 0.5,
                                op0=AL.mult, op1=AL.add)

        # ---- E1^T = sig^T @ Ve^T directly ; softmax over free dim
        E1Tp = sml.tile([T, T], f32, tag="sml", name="e1t")
        nc.tensor.matmul(E1Tp[:], sig[:], VeT[:], start=True, stop=True)
        E1Ts = sg.tile([T, T], bf16, tag="e1ts")
        nc.vector.tensor_copy(E1Ts[:], E1Tp[:])
        # values are O(1e-1): skip the max-subtraction for softmax
        sume = sg.tile([T, 1], f32, tag="sume")
        EtT = sg.tile([T, T], bf16, tag="ett")
        nc.scalar.activation(EtT[:], E1Ts[:], AF.Exp,
                             scale=1.0, accum_out=sume[:, 0:1])
        rse = sg.tile([T, 1], f32, tag="rse")
        nc.vector.reciprocal(rse[:], sume[:])
        nc.vector.tensor_scalar(EtT[:], EtT[:], rse[:, 0:1], None, op0=AL.mult)
        Etp = hlf.tile([T, T], bf16, tag="hlf", name="etp")
        nc.tensor.transpose(Etp[:], EtT[:], I128b[0:16, 0:16])
        Et = sg.tile([T, T], bf16, tag="et")
        nc.vector.tensor_copy(Et[:], Etp[:])

        # ---- w1e row (1,16) = Ws1.T @ EtT ; broadcast to (128,16)
        w1p = sml.tile([1, T], f32, tag="sml", name="w1p")
        nc.tensor.matmul(w1p[:], Ws1[:], EtT[:], start=True, stop=True)
        w1row = sg.tile([1, T], bf16, tag="w1row")
        nc.scalar.copy(w1row[:], w1p[:])
        w1Bp = sml.tile([P, T], f32, tag="sml", name="w1bp")
        nc.tensor.matmul(w1Bp[:], ones1[:], w1row[:], start=True, stop=True)
        w1B = sg.tile([P, T], bf16, tag="w1b")
        nc.vector.tensor_copy(w1B[:], w1Bp[:])

        # ---- w1Bpair[p=(v,f), s] = w1e[2s+v]
        w1Bp2 = sg.tile([P, 8], bf16, tag="w1bp2")
        nc.vector.tensor_copy(w1Bp2[0:64, :], w1B[0:64, 0:T:2])
        nc.vector.tensor_copy(w1Bp2[64:128, :], w1B[64:128, 1:T:2])
        # ---- Ws2wP[p=(v,f), s, t] = Ws2d[p,t] * w1e[2s+v]   (128, 8, 16)
        Ws2w = sg.tile([P, 8, T], bf16, tag="ws2w")
        nc.vector.tensor_tensor(
            Ws2w[:],
            Ws2d[:].unsqueeze(1).broadcast_to((P, 8, T)),
            w1Bp2[:].unsqueeze(2).broadcast_to((P, 8, T)),
            op=AL.mult)

        # ---- lhs_sT (16, 512) = sum_t1 (Ws2*w1e[t1]).T @ X^T[t1]
        lsTp = sml.tile([T, N], f32, tag="sml", name="lst")
        for s in range(8):
            nc.tensor.matmul(lsTp[:], Ws2w[:, s, :], Xw[s][:, :],
                             start=(s == 0), stop=(s == 7))
        lsT = sg.tile([T, N], bf16, tag="lsts")
        nc.scalar.copy(lsT[:], lsTp[:])

        # ---- rhs_s (16, 512) = Et-weighted rhs3
        rsp = sml.tile([T, N], f32, tag="sml", name="rsp")
        nc.tensor.matmul(rsp[:], Et[:], R48[0:16, :], start=True, stop=True)
        rss = sg.tile([T, N], bf16, tag="rss")
        nc.scalar.copy(rss[:], rsp[:])

        # ---- P chunks + G = sigmoid(P + bs) = 0.5*tanh(0.5*P + bsh) + 0.5
        for k in range(CH):
            Pp = big.tile([P, N], f32, tag="big", name="pp")
            nc.tensor.matmul(Pp[:], lsT[:, k * P:(k + 1) * P], rss[:],
                             start=True, stop=True)
            nc.vector.scalar_tensor_tensor(G[k][:], Pp[:], 0.5, bsh[k][:],
                                           op0=AL.mult, op1=AL.add)
            nc.scalar.activation(G[k][:], G[k][:], AF.Tanh)
            nc.vector.tensor_scalar(G[k][:], G[k][:], 0.5, 0.5,
                                    op0=AL.mult, op1=AL.add)

        # ---- M1T chunks (c-part, r) + masked softmax -> A1T, dS
        for c in range(CH):
            Mp = big.tile([P, N], f32, tag="big", name="mp")
            for k in range(CH):
                nc.tensor.matmul(Mp[:], G[k][:, c * P:(c + 1) * P], VsT[k][:],
                                 start=(k == 0), stop=(k == CH - 1))
            sme = sg.tile([P, 1], f32, tag=f"sme{c}", name=f"sme{c}")
            nc.scalar.activation(Ex[c][:], Mp[:], AF.Exp,
                                 scale=1.0, accum_out=sme[:, 0:1])
            rcp = sg.tile([P, 1], f32, tag=f"rcp{c}", name=f"rcp{c}")
            nc.vector.reciprocal(rcp[:], sme[:])
            # A1T = (Ex * rcp) * WT   (= S^T o W^T)
            nc.vector.scalar_tensor_tensor(A1T[c][:], Ex[c][:], rcp[:, 0:1],
                                           WT[c][:], op0=AL.mult, op1=AL.mult)
            # diag: dS = sum_r (Ex*rcp)*I over the diagonal block
            dtmp = sg.tile([P, P], bf16, tag="dtmp")
            nc.vector.scalar_tensor_tensor(dtmp[:], Ex[c][:, c * P:(c + 1) * P],
                                           rcp[:, 0:1], I128b[:],
                                           op0=AL.mult, op1=AL.mult)
            nc.vector.tensor_reduce(dSv[c][:], dtmp[:], axis=AX.X, op=AL.add)

        # ---- dS row + broadcast tile (128, 512)
        dSrp = sml.tile([1, N], f32, tag="sml", name="dsrp")
        for c in range(CH):
            nc.tensor.transpose(dSrp[:, c * P:(c + 1) * P], dSv[c][:], I128f[:])
        dSrow = sg.tile([1, N], bf16, tag="dsrow")
        nc.scalar.copy(dSrow[:], dSrp[:])
        dSBp = sml.tile([P, N], f32, tag="sml", name="dsbp")
        nc.tensor.matmul(dSBp[:], ones1[:], dSrow[:], start=True, stop=True)
        nc.scalar.copy(dSB[:], dSBp[:])

        # ---- Tx0 in n-layout (all t at once)
        for k in range(CH):
            nc.vector.tensor_scalar(Tx0n[k][:], Xn[k][:], dSv[k][:, 0:1], None,
                                    op0=AL.mult)

        # =====================================================
        # Cheb + conv + LN, software-pipelined per pair
        # =====================================================
        Tx0P = {}
        TAp = {}
        Tx1T = {}
        ptA = {}
        Tx1n = {}
        TBp = {}
        Tx2T = {}
        TCp = {}
        XhP = {-1: zerot, NP: zerot}
        TDp = {}
        ZT = {}

        def e_tx0p(q):
            t = txp.tile([P, N], bf16, tag="tx0p", name=f"tx0p{q}")
            nc.gpsimd.tensor_tensor(t[:], Xw[q][:], dSB[:], op=AL.mult)
            Tx0P[q] = t

        def e_ta(q):
            p = big.tile([P, N], f32, tag="big", name=f"ta{q}")
            for k in range(CH):
                lhs = Tx0n[k][:, 2 * q * F:(2 * q + 2) * F]
                nc.tensor.matmul(p[:], lhs, A1T[k][:],
                                 start=(k == 0), stop=(k == CH - 1))
            TAp[q] = p

        def e_b(q):
            t = txp.tile([P, N], bf16, tag="tx1t", name=f"tx1t{q}")
            nc.vector.tensor_copy(t[:], TAp[q][:])
            Tx1T[q] = t

        def e_c(q):
            p = hlf.tile([P, N], bf16, tag="hlf", name=f"pta{q}")
            for k in range(CH):
                nc.tensor.transpose(p[:, k * P:(k + 1) * P],
                                    Tx1T[q][:, k * P:(k + 1) * P], I128b[:])
            ptA[q] = p

        def e_d(q):
            t = txp.tile([P, N], bf16, tag="tx1n", name=f"tx1n{q}")
            nc.scalar.copy(t[:], ptA[q][:])
            Tx1n[q] = t

        def e_e(q):
            p = big.tile([P, N], f32, tag="big", name=f"tb{q}")
            for k in range(CH):
                nc.tensor.matmul(p[:], Tx1n[q][:, k * P:(k + 1) * P], WT[k][:],
                                 start=(k == 0), stop=(k == CH - 1))
            TBp[q] = p

        def e_f(q):
            t = txp.tile([P, N], bf16, tag="tx2t", name=f"tx2t{q}")
            nc.vector.scalar_tensor_tensor(t[:], TBp[q][:], 2.0, Tx0P[q][:],
                                           op0=AL.mult, op1=AL.subtract)
            Tx2T[q] = t

        def e_g(q):
            p = big.tile([P, N], f32, tag="big", name=f"tc{q}")
            nc.tensor.matmul(p[:], WcP[0][:], Tx0P[q][:], start=True, stop=False)
            nc.tensor.matmul(p[:], WcP[1][:], Tx1T[q][:], start=False, stop=False)
            nc.tensor.matmul(p[:], WcP[2][:], Tx2T[q][:], start=False, stop=True)
            TCp[q] = p

        def e_h(q):
            t = xhp.tile([P, N], bf16, tag="xh", name=f"xh{q}")
            nc.scalar.activation(t[:], TCp[q][:], AF.Relu, bias=bch[:, 0:1],
                                 scale=1.0)
            XhP[q] = t

        def e_i(q):
            p = big.tile([P, N], f32, tag="big", name=f"td{q}")
            nc.tensor.matmul(p[:], Lprev[:], XhP[q - 1][:], start=True, stop=False)
            nc.tensor.matmul(p[:], Lmid[:], XhP[q][:], start=False, stop=False)
            nc.tensor.matmul(p[:], Lnext[:], XhP[q + 1][:], start=False, stop=False)
            nc.tensor.matmul(p[:], WrP[:], Xw[q][:], start=False, stop=True)
            TDp[q] = p

        def e_j(q):
            t = lnp.tile([P, N], bf16, tag="zt", name=f"zt{q}")
            nc.scalar.activation(t[:], TDp[q][:], AF.Relu, bias=btr[:, 0:1],
                                 scale=1.0)
            ZT[q] = t

        def e_ln(q):
            z = ZT[q]
            sq = lnp.tile([P, N], bf16, tag="sq", name=f"sq{q}")
            nc.vector.tensor_tensor(sq[:], z[:], z[:], op=AL.mult)
            # stats: rows 0:2 mean per v, rows 32:34 E[x^2] per v (B2 = 1/64)
            # (matmul out base partition must be 0/32/64)
            s12 = sml.tile([34, N], f32, tag="sml", name=f"s12{q}")
            nc.tensor.matmul(s12[0:2, :], B2[:], z[:], start=True, stop=True)
            nc.tensor.matmul(s12[32:34, :], B2[:], sq[:], start=True, stop=True)
            mu = lnp.tile([2, N], f32, tag="mu", name=f"mu{q}")
            nc.scalar.copy(mu[:], s12[0:2, :])
            mu2 = lnp.tile([2, N], f32, tag="mu2", name=f"mu2{q}")
            nc.vector.tensor_tensor(mu2[:], mu[:], mu[:], op=AL.mult)
            # var+eps = (msq + eps) - mu^2 in one stt
            var = lnp.tile([2, N], f32, tag="var", name=f"var{q}")
            nc.vector.scalar_tensor_tensor(var[:], s12[32:34, :], LN_EPS, mu2[:],
                                           op0=AL.add, op1=AL.subtract)
            # rstd = sqrt(1/(var+eps)); approx recip is ~18 bits, plenty
            rv = lnp.tile([2, N], f32, tag="rv", name=f"rv{q}")
            nc.vector.reciprocal_approx_fast(rv[:], var[:])
            rstd = lnp.tile([2, N], bf16, tag="rstd", name=f"rstd{q}")
            nc.scalar.activation(rstd[:], rv[:], AF.Sqrt)
            # nmr = mu * rstd
            nmr = lnp.tile([2, N], bf16, tag="nmr", name=f"nmr{q}")
            nc.gpsimd.tensor_tensor(nmr[:], mu[:], rstd[:], op=AL.mult)
            # broadcast to 128 partitions
            rBp = big.tile([P, N], f32, tag="big", name=f"rbp{q}")
            nc.tensor.matmul(rBp[:], B2T[:], rstd[:], start=True, stop=True)
            nBp = big.tile([P, N], f32, tag="big", name=f"nbp{q}")
            nc.tensor.matmul(nBp[:], B2T[:], nmr[:], start=True, stop=True)
            # w = (z*rB)*gam - (nB*gam - bet)
            u = lnp.tile([P, N], bf16, tag="u", name=f"u{q}")
            nc.vector.tensor_tensor(u[:], z[:], rBp[:], op=AL.mult)
            nB2 = lnp.tile([P, N], bf16, tag="nb2", name=f"nb2{q}")
            nc.scalar.activation(nB2[:], nBp[:], AF.Identity,
                                 bias=nbetP[:, 0:1], scale=gamP[:, 0:1])
            w = lnp.tile([P, N], bf16, tag="w", name=f"w{q}")
            nc.vector.scalar_tensor_tensor(w[:], u[:], gamP[:, 0:1], nB2[:],
                                           op0=AL.mult, op1=AL.subtract)
            nc.sync.dma_start(out=ZoutD[q * P:(q + 1) * P, :], in_=w[:])

        # pipeline drive
        e_tx0p(0)
        e_ta(0)
        e_b(0)
        e_tx0p(1)
        e_ta(1)
        e_c(0)
        e_d(0)
        e_e(0)
        e_f(0)
        e_g(0)
        e_h(0)
        e_b(1)
        e_tx0p(2)
        for q in range(2, NP):
            e_ta(q)
            e_c(q - 1)
            e_d(q - 1)
            e_e(q - 1)
            e_f(q - 1)
            e_g(q - 1)
            e_h(q - 1)
            e_b(q)
            if q + 1 < NP:
                e_tx0p(q + 1)
            e_i(q - 2)
            e_j(q - 2)
            e_ln(q - 2)
        e_c(NP - 1)
        e_d(NP - 1)
        e_e(NP - 1)
        e_f(NP - 1)
        e_g(NP - 1)
        e_h(NP - 1)
        e_i(NP - 2)
        e_j(NP - 2)
        e_ln(NP - 2)
        e_i(NP - 1)
        e_j(NP - 1)
        e_ln(NP - 1)

    nc.compile()
    return nc


def _host_prep(inputs):
    import ml_dtypes
    bf = ml_dtypes.bfloat16

    X = np.asarray(inputs['X'], np.float32)
    edge_index = np.asarray(inputs['edge_index'])
    U1 = np.asarray(inputs['U1'], np.float32)
    U2 = np.asarray(inputs['U2'], np.float32)
    U3 = np.asarray(inputs['U3'], np.float32)
    be = np.asarray(inputs['be'], np.float32)
    Ve = np.asarray(inputs['Ve'], np.float32)
    Ws1 = np.asarray(inputs['Ws1'], np.float32)
    Ws2 = np.asarray(inputs['Ws2'], np.float32)
    Ws3 = np.asarray(inputs['Ws3'], np.float32)
    bs = np.asarray(inputs['bs'], np.float32)
    Vs = np.asarray(inputs['Vs'], np.float32)
    W_cheb = np.asarray(inputs['W_cheb'], np.float32)
    b_cheb = np.asarray(inputs['b_cheb'], np.float32)
    Wt = np.asarray(inputs['Wt'], np.float32)
    bt = np.asarray(inputs['bt'], np.float32)
    Wr = np.asarray(inputs['Wr'], np.float32)
    br = np.asarray(inputs['br'], np.float32)
    gamma = np.asarray(inputs['gamma'], np.float32)
    beta = np.asarray(inputs['beta'], np.float32)

    # dense symmetric-norm matrix (self-loop +I/-I terms cancel)
    row, col = edge_index[0].astype(np.int64), edge_index[1].astype(np.int64)
    deg = np.zeros(N, np.float32)
    np.add.at(deg, row, 1.0)
    dis = np.where(deg > 0, 1.0 / np.sqrt(np.maximum(deg, 1.0)), 0.0).astype(np.float32)
    wn = -dis[row] * dis[col]
    W = np.zeros((N, N), np.float32)
    np.add.at(W, (row, col), wn)

    # conv block matrices: L[(v,fi),(u,fo)] = Wt[fo,fi,0,dt]
    WtT = [np.ascontiguousarray(Wt[:, :, 0, d].T) for d in range(3)]  # (fi,fo)
    Z64 = np.zeros((F, F), np.float32)
    Lmid = np.block([[WtT[1], WtT[0]], [WtT[2], WtT[1]]]).astype(bf)
    Lprev = np.block([[Z64, Z64], [WtT[0], Z64]]).astype(bf)
    Lnext = np.block([[Z64, WtT[2]], [Z64, Z64]]).astype(bf)
    WrT = np.ascontiguousarray(Wr[:, :, 0, 0].T)
    WrP = np.block([[WrT, Z64], [Z64, WrT]]).astype(bf)
    WcP = np.stack([np.block([[W_cheb[k], Z64], [Z64, W_cheb[k]]]) for k in range(3)]
                   ).astype(bf)

    Wpk = np.stack([WcP[0], WcP[1], WcP[2], Lprev, Lmid, Lnext, WrP])

    Pf = np.zeros((P, PFW), np.float32)
    Pf[:, 0] = np.tile(gamma, 2)
    Pf[:, 1] = np.tile(-beta, 2)
    Pf[:, 2] = np.tile(b_cheb, 2)
    Pf[:, 3] = np.tile(bt + br, 2)
    Pf[:, 4:132] = np.eye(P, dtype=np.float32)

    shared = {
        'bsh': (0.5 * bs[0]).astype(bf),
        'VsT': np.ascontiguousarray(Vs.T).astype(bf),
        'WT': np.ascontiguousarray(W.T).astype(bf),
        'Wpk': Wpk,
    }

    in_maps = []
    for core in range(8):
        b, h = core // 2, core % 2
        tmap = list(range(16)) if h == 0 else list(range(6, 16)) + list(range(6))
        Xp = X[b][:, :, tmap]                              # (N, F, 16)
        Xn = np.ascontiguousarray(Xp.transpose(0, 2, 1).reshape(N, T * F)).astype(bf)
        Xw = np.ascontiguousarray(Xp.transpose(2, 1, 0).reshape(8, P, N)).astype(bf)
        UW = np.zeros((8, P, 48), np.float32)
        for tp in range(16):
            s, v = tp // 2, tp % 2
            UW[s, 64 * v:64 * v + 64, tp] = Ws3
            UW[s, 64 * v:64 * v + 64, 32 + tp] = U3
        Pb = np.zeros((P, PBW), np.float32)
        Pb[:, 0:4] = U1.reshape(4, P).T
        Pb[:, 4:20] = np.vstack([Ws2, Ws2])
        Pb[0:16, 20:36] = Ve[np.ix_(tmap, tmap)].T
        Pb[0:16, 36] = Ws1[tmap]
        Pb[0, 37:165] = 1.0
        Pb[:, 165:293] = np.eye(P, dtype=np.float32)
        Pb[0:64, 293:805] = U2
        Pb[0, 805:1061] = np.eye(T, dtype=np.float32).reshape(-1)
        # B2: (128, 2) block indicator * 1/64 for per-v mean over f
        Pb[0:64, 1061] = 1.0 / 64
        Pb[64:128, 1062] = 1.0 / 64
        # B2T: (2, 128) block indicator for broadcast back
        Pb[0, 1063:1127] = 1.0
        Pb[1, 1127:1191] = 1.0
        Pfc = Pf.copy()
        Pfc[0:16, 132:148] = 0.5 * be[0][np.ix_(tmap, tmap)]
        m = dict(shared)
        m.update({
            'Xn': Xn, 'Xw': Xw, 'UW': UW.astype(bf),
            'Pb': Pb.astype(bf), 'Pf': Pfc,
        })
        in_maps.append(m)
    return in_maps


def kernel(**inputs):
    import sys
    if '/opt/trn_rl_repo' not in sys.path:
        sys.path.insert(0, '/opt/trn_rl_repo')
    from concourse.bass_utils import run_bass_kernel_spmd

    if 'nc' not in _CACHE:
        _CACHE['nc'] = _build_program()
    nc = _CACHE['nc']

    in_maps = _host_prep(inputs)
    res = run_bass_kernel_spmd(nc, in_maps, list(range(8)))
    out = np.zeros((B, N, F, T), np.float32)
    for core in range(8):
        b, h = core // 2, core % 2
        Z = np.asarray(res.results[core]['Zout']).astype(np.float32)
        # rows q*128 + v*64 + f, cols n  ->  (n, f, slot=2q+v)
        Zs = Z.reshape(NP, 2, F, N).transpose(3, 2, 0, 1).reshape(N, F, NSLOT)
        wstart = 0 if h == 0 else 6
        jlo = 0 if h == 0 else 2
        out[b, :, :, wstart + jlo:wstart + jlo + 8] = Zs[:, :, jlo:jlo + 8]
    return out


# revision 24
# speedup vs baseline: 1.0808x; 1.0148x over previous
"""ASTGCN block Trainium2 kernel (v2).

Strategy: 8 cores; core c handles batch b = c//2, time-half h = c%2 (8 output
timesteps each, data-parallel over B and T). Attention (temporal Et, spatial
S) is per-b and replicated on the 2 cores sharing a b. The sparse graph
propagation is reformulated as dense (N,N) matmuls: the edge-scatter of the
symmetric norm is accumulated host-side into a dense W (the +I/-I self-loop
terms cancel), so  prop1(h) = (W*S) @ h  and  prop2(h) = W @ h.

v2 changes vs baseline:
- Input DMAs ordered by first use (Pb/Pf/Xn first) and X tensors split in
  halves so attention matmuls start ~5us in instead of after all loads.
- Single activation-table regime: sigmoid via tanh (0.5*tanh(x/2)+0.5, in
  the exp table) and LN rstd via exp(-0.5*ln(var+eps)); only one table
  switch in the whole program (exp_and_others -> natural_log_exp...).
- LayerNorm runs in pair layout: per-pair stats via ones-block matmuls on
  PE (reduce over the f partition rows), rstd/-mu*rstd broadcast back with
  block matmuls; no transposes of the conv output at all.
- Output stored in pair layout as bf16; host does the final (f,n) -> (n,f)
  transpose and fp32 upcast.
- cheb -> conv -> LN -> store software-pipelined across the 5 timestep
  pairs to keep PE dense (p-state) and overlap store DMAs with compute.

Per-core time axis is PERMUTED so the program is identical SPMD: slot t' maps
to global t via tmap (identity for h=0, rotated by 6 for h=1); all
t-dependent weights (be, Ve, Ws1, UW) are permuted host-side to match.
"""

import numpy as np

B, N, F, T = 4, 512, 64, 16
P = 128
CH = N // P            # 4 n-chunks
NSLOT = 10             # cheb window timesteps per core (5 pairs)
NP = NSLOT // 2        # 5 pairs
LN_EPS = 1e-5

PBW = 1719             # packed bf16 constant width
PFW = 148              # packed f32 constant width

_CACHE = {}


def _build_program():
    import sys
    if '/opt/trn_rl_repo' not in sys.path:
        sys.path.insert(0, '/opt/trn_rl_repo')
    from contextlib import ExitStack
    import concourse.bass as bass
    import concourse.tile as tile
    from concourse import bacc, mybir

    dt = mybir.dt
    AL = mybir.AluOpType
    AF = mybir.ActivationFunctionType
    AX = mybir.AxisListType
    f32 = dt.float32
    bf16 = dt.bfloat16

    nc = bacc.Bacc("TRN2", target_bir_lowering=False, debug=False, num_devices=1)

    def din(name, shape, d=bf16):
        return nc.dram_tensor(name, list(shape), d, kind="ExternalInput").ap()

    XnD   = din("Xn", (N, T * F))
    XwD   = din("Xw", (8, P, N))
    UWD   = din("UW", (8, P, 48))
    bshD  = din("bsh", (N, N))          # 0.5 * bs
    VsTD  = din("VsT", (N, N))
    WTD   = din("WT", (N, N))
    WpkD  = din("Wpk", (7, P, P))
    PbD   = din("Pb", (P, PBW))
    PfD   = din("Pf", (P, PFW), f32)
    ZoutD = nc.dram_tensor("Zout", [NP * P, N], bf16, kind="ExternalOutput").ap()

    with tile.TileContext(nc) as tc, ExitStack() as ctx:
        sg = ctx.enter_context(tc.tile_pool(name="sg", bufs=1))
        big = ctx.enter_context(tc.tile_pool(name="big", bufs=5, space="PSUM"))
        sml = ctx.enter_context(tc.tile_pool(name="sml", bufs=2, space="PSUM"))
        hlf = ctx.enter_context(tc.tile_pool(name="hlf", bufs=1, space="PSUM"))
        xhp = ctx.enter_context(tc.tile_pool(name="xhp", bufs=7))
        txp = ctx.enter_context(tc.tile_pool(name="txp", bufs=3))
        lnp = ctx.enter_context(tc.tile_pool(name="lnp", bufs=3))

        # ------------- input DMAs, ordered by first use -------------
        Pb = sg.tile([P, PBW], bf16, tag="pb")
        nc.sync.dma_start(out=Pb[:], in_=PbD)
        Pf = sg.tile([P, PFW], f32, tag="pf")
        nc.sync.dma_start(out=Pf[:], in_=PfD)
        XnA = sg.tile([P, 2, T * F], bf16, tag="xna")
        XnB = sg.tile([P, 2, T * F], bf16, tag="xnb")
        XnDr = XnD.rearrange("(k p) t -> p k t", k=CH)
        nc.sync.dma_start(out=XnA[:], in_=XnDr[:, 0:2, :])
        nc.sync.dma_start(out=XnB[:], in_=XnDr[:, 2:4, :])
        UWAll = sg.tile([P, 8, 48], bf16, tag="uwall")
        nc.sync.dma_start(out=UWAll[:], in_=UWD.rearrange("s p n -> p s n"))
        XwA = sg.tile([P, 4, N], bf16, tag="xwa")
        XwB = sg.tile([P, 4, N], bf16, tag="xwb")
        XwDr = XwD.rearrange("s p n -> p s n")
        nc.sync.dma_start(out=XwA[:], in_=XwDr[:, 0:4, :])
        nc.sync.dma_start(out=XwB[:], in_=XwDr[:, 4:8, :])
        bsAll = sg.tile([P, CH, N], bf16, tag="bsall")
        nc.sync.dma_start(out=bsAll[:], in_=bshD.rearrange("(k p) n -> p k n", k=CH))
        VsTAll = sg.tile([P, CH, N], bf16, tag="vstall")
        nc.sync.dma_start(out=VsTAll[:], in_=VsTD.rearrange("(k p) n -> p k n", k=CH))
        WTAll = sg.tile([P, CH, N], bf16, tag="wtall")
        nc.sync.dma_start(out=WTAll[:], in_=WTD.rearrange("(k p) n -> p k n", k=CH))
        Wpk = sg.tile([P, 7, P], bf16, tag="wpk")
        nc.sync.dma_start(out=Wpk[:], in_=WpkD.rearrange("w p c -> p w c"))

        Xn = [XnA[:, 0, :], XnA[:, 1, :], XnB[:, 0, :], XnB[:, 1, :]]
        Xw = [XwA[:, s, :] for s in range(4)] + [XwB[:, s, :] for s in range(4)]
        UW = [UWAll[:, s, :] for s in range(8)]
        bsh = [bsAll[:, k, :] for k in range(CH)]
        VsT = [VsTAll[:, k, :] for k in range(CH)]
        WT = [WTAll[:, k, :] for k in range(CH)]
        WcP = [Wpk[:, k, :] for k in range(3)]
        Lprev, Lmid, Lnext, WrP = (Wpk[:, 3, :], Wpk[:, 4, :], Wpk[:, 5, :],
                                   Wpk[:, 6, :])
        # packed bf16 layout
        U1r = Pb[:, 0:4]
        Ws2d = Pb[:, 4:20]
        VeT = Pb[0:16, 20:36]
        Ws1 = Pb[0:16, 36:37]
        ones1 = Pb[0:1, 37:165]
        I128b = Pb[:, 165:293]
        U2 = Pb[0:64, 293:805]
        I16r = Pb[0:1, 805:1061]     # I16 rows flattened: e_t = [0:1, 16t:16t+16]
        B2 = Pb[:, 1061:1063]        # (128,2) block col-indicator * 1/64
        B2T = Pb[0:2, 1063:1191]     # (2,128) block row-indicator
        hcVe = Pb[0:1, 1191:1207]    # 0.5*colsum(VeT')  [sigmoid-fold row]
        vch = Pb[0:1, 1207:1719]     # 0.5*colsum(VsT')  [sigmoid-fold row]
        # packed f32 layout
        gamP = Pf[:, 0:1]
        nbetP = Pf[:, 1:2]           # -beta
        bch = Pf[:, 2:3]
        btr = Pf[:, 3:4]
        I128f = Pf[:, 4:132]
        bePh = Pf[0:16, 132:148]     # 0.5 * be (permuted)

        zerot = sg.tile([P, N], bf16, tag="zerot")
        nc.vector.memset(zerot[:], 0.0)
        epsP = sg.tile([P, 1], f32, tag="epsP")
        nc.vector.memset(epsP[:], LN_EPS)

        # persistent sbuf intermediates
        G = [sg.tile([P, N], bf16, tag=f"g{k}", name=f"g{k}") for k in range(CH)]
        Ex = [sg.tile([P, N], bf16, tag=f"ex{k}", name=f"ex{k}") for k in range(CH)]
        A1T = [sg.tile([P, N], bf16, tag=f"a1t{k}", name=f"a1t{k}") for k in range(CH)]
        dSv = [sg.tile([P, 1], f32, tag=f"dsv{k}", name=f"dsv{k}") for k in range(CH)]
        Tx0n = [sg.tile([P, T * F], bf16, tag=f"tx0n{k}", name=f"tx0n{k}")
                for k in range(CH)]
        dSB = sg.tile([P, N], bf16, tag="dsb")

        # =====================================================
        # Attention phase
        # =====================================================
        # ---- lhs0[(t,f)] = sum_n U1[n] X[n,(t,f)]  -> (1,1024)
        L0a = sml.tile([1, 512], f32, tag="sml", name="l0a")
        for k in range(CH):
            nc.tensor.matmul(L0a[:], U1r[:, k:k + 1], Xn[k][:, 0:512],
                             start=(k == 0), stop=(k == CH - 1))
        L0b = sml.tile([1, 512], f32, tag="sml", name="l0b")
        for k in range(CH):
            nc.tensor.matmul(L0b[:], U1r[:, k:k + 1], Xn[k][:, 512:1024],
                             start=(k == 0), stop=(k == CH - 1))
        lhs0row = sg.tile([1, T * F], bf16, tag="lhs0row")
        nc.vector.tensor_copy(lhs0row[:, 0:512], L0a[:])
        nc.vector.tensor_copy(lhs0row[:, 512:1024], L0b[:])
        # reshape to (64,16) via 16 rank-1 matmuls against identity rows
        l0Fp = sml.tile([F, T], f32, tag="sml", name="l0fp")
        for t in range(T):
            nc.tensor.matmul(l0Fp[:], lhs0row[0:1, 64 * t:64 * t + 64],
                             I16r[0:1, 16 * t:16 * t + 16],
                             start=(t == 0), stop=(t == T - 1))
        lhs0F = sg.tile([F, T], bf16, tag="lhs0f")
        nc.vector.tensor_copy(lhs0F[:], l0Fp[:])

        # ---- lhs2T chunks (n,16) = U2[:,chunk].T @ lhs0F
        lhs2T = []
        for k in range(CH):
            pt = sml.tile([P, T], f32, tag="sml", name="l2t")
            nc.tensor.matmul(pt[:], U2[:, k * P:(k + 1) * P], lhs0F[:],
                             start=True, stop=True)
            st = sg.tile([P, T], bf16, tag=f"l2ts{k}", name=f"l2ts{k}")
            nc.vector.tensor_copy(st[:], pt[:])
            lhs2T.append(st)

        # ---- R48: rows 0:16 rhs3T (Ws3), rows 32:48 rhs_tT (U3)
        R48p = sml.tile([48, N], f32, tag="sml", name="r48")
        for s in range(8):
            nc.tensor.matmul(R48p[:], UW[s][:, :], Xw[s][:, :],
                             start=(s == 0), stop=(s == 7))
        R48 = sg.tile([48, N], bf16, tag="r48s")
        nc.scalar.copy(R48[:], R48p[:])

        # ---- rhs_tn chunks: transpose R48[32:48]
        rhs_tn = []
        for k in range(CH):
            pt = hlf.tile([P, T], bf16, tag="hlf", name="rtn")
            nc.tensor.transpose(pt[:], R48[32:48, k * P:(k + 1) * P],
                                I128b[32:48, 32:48])
            st = sg.tile([P, T], bf16, tag=f"rtns{k}", name=f"rtns{k}")
            nc.vector.tensor_copy(st[:], pt[:])
            rhs_tn.append(st)

        # ---- P0 (16,16) = lhs_t @ rhs_t ; sigmoid via tanh:
        # sig = 0.5*tanh(0.5*(P0+be)) + 0.5
        P0p = sml.tile([T, T], f32, tag="sml", name="p0")
        for k in range(CH):
            nc.tensor.matmul(P0p[:], lhs2T[k][:], rhs_tn[k][:],
                             start=(k == 0), stop=(k == CH - 1))
        # sigmoid fold: sig = 0.5*tanh(0.5*(P0+be)) + 0.5; the affine is folded
        # into the E1T matmul (VeT is 0.5-scaled host-side, hcVe rank-1 term)
        sig = sg.tile([T, T], bf16, tag="sig")
        nc.vector.scalar_tensor_tensor(sig[:], P0p[:], 0.5, bePh[:],
                                       op0=AL.mult, op1=AL.add)
        nc.scalar.activation(sig[:], sig[:], AF.Tanh)

        # ---- E1^T = sigmoid^T @ Ve^T ; softmax over free dim
        E1Tp = sml.tile([T, T], f32, tag="sml", name="e1t")
        nc.tensor.matmul(E1Tp[:], sig[:], VeT[:], start=True, stop=False)
        nc.tensor.matmul(E1Tp[:], ones1[0:1, 0:16], hcVe[:],
                         start=False, stop=True)
        E1Ts = sg.tile([T, T], bf16, tag="e1ts")
        nc.vector.tensor_copy(E1Ts[:], E1Tp[:])
        # values are O(1e-1): skip the max-subtraction for softmax
        sume = sg.tile([T, 1], f32, tag="sume")
        EtT = sg.tile([T, T], bf16, tag="ett")
        nc.scalar.activation(EtT[:], E1Ts[:], AF.Exp,
                             scale=1.0, accum_out=sume[:, 0:1])
        rse = sg.tile([T, 1], f32, tag="rse")
        nc.vector.reciprocal(rse[:], sume[:])
        nc.vector.tensor_scalar(EtT[:], EtT[:], rse[:, 0:1], None, op0=AL.mult)
        Etp = hlf.tile([T, T], bf16, tag="hlf", name="etp")
        nc.tensor.transpose(Etp[:], EtT[:], I128b[0:16, 0:16])
        Et = sg.tile([T, T], bf16, tag="et")
        nc.vector.tensor_copy(Et[:], Etp[:])

        # ---- w1e row (1,16) = Ws1.T @ EtT ; broadcast to (128,16)
        w1p = sml.tile([1, T], f32, tag="sml", name="w1p")
        nc.tensor.matmul(w1p[:], Ws1[:], EtT[:], start=True, stop=True)
        w1row = sg.tile([1, T], bf16, tag="w1row")
        nc.scalar.copy(w1row[:], w1p[:])
        w1Bp = sml.tile([P, T], f32, tag="sml", name="w1bp")
        nc.tensor.matmul(w1Bp[:], ones1[:], w1row[:], start=True, stop=True)
        w1B = sg.tile([P, T], bf16, tag="w1b")
        nc.vector.tensor_copy(w1B[:], w1Bp[:])

        # ---- w1Bpair[p=(v,f), s] = w1e[2s+v]
        w1Bp2 = sg.tile([P, 8], bf16, tag="w1bp2")
        nc.vector.tensor_copy(w1Bp2[0:64, :], w1B[0:64, 0:T:2])
        nc.vector.tensor_copy(w1Bp2[64:128, :], w1B[64:128, 1:T:2])
        # ---- Ws2wP[p=(v,f), s, t] = Ws2d[p,t] * w1e[2s+v]   (128, 8, 16)
        Ws2w = sg.tile([P, 8, T], bf16, tag="ws2w")
        nc.vector.tensor_tensor(
            Ws2w[:],
            Ws2d[:].unsqueeze(1).broadcast_to((P, 8, T)),
            w1Bp2[:].unsqueeze(2).broadcast_to((P, 8, T)),
            op=AL.mult)

        # ---- lhs_sT (16, 512) = sum_t1 (Ws2*w1e[t1]).T @ X^T[t1]
        lsTp = sml.tile([T, N], f32, tag="sml", name="lst")
        for s in range(8):
            nc.tensor.matmul(lsTp[:], Ws2w[:, s, :], Xw[s][:, :],
                             start=(s == 0), stop=(s == 7))
        lsT = sg.tile([T, N], bf16, tag="lsts")
        nc.scalar.copy(lsT[:], lsTp[:])

        # ---- rhs_s (16, 512) = Et-weighted rhs3
        rsp = sml.tile([T, N], f32, tag="sml", name="rsp")
        nc.tensor.matmul(rsp[:], Et[:], R48[0:16, :], start=True, stop=True)
        rss = sg.tile([T, N], bf16, tag="rss")
        nc.scalar.copy(rss[:], rsp[:])

        # ---- P chunks; G holds tanh(0.5*P + bsh); the sigmoid affine is
        # folded into M1T (VsT 0.5-scaled host-side + vch rank-1 term)
        for k in range(CH):
            Pp = big.tile([P, N], f32, tag="big", name="pp")
            nc.tensor.matmul(Pp[:], lsT[:, k * P:(k + 1) * P], rss[:],
                             start=True, stop=True)
            nc.vector.scalar_tensor_tensor(G[k][:], Pp[:], 0.5, bsh[k][:],
                                           op0=AL.mult, op1=AL.add)
            nc.scalar.activation(G[k][:], G[k][:], AF.Tanh)

        # ---- M1T chunks (c-part, r) + masked softmax -> A1T, dS
        for c in range(CH):
            Mp = big.tile([P, N], f32, tag="big", name="mp")
            for k in range(CH):
                nc.tensor.matmul(Mp[:], G[k][:, c * P:(c + 1) * P], VsT[k][:],
                                 start=(k == 0), stop=False)
            nc.tensor.matmul(Mp[:], ones1[0:1, 0:128], vch[:],
                             start=False, stop=True)
            sme = sg.tile([P, 1], f32, tag=f"sme{c}", name=f"sme{c}")
            nc.scalar.activation(Ex[c][:], Mp[:], AF.Exp,
                                 scale=1.0, accum_out=sme[:, 0:1])
            rcp = sg.tile([P, 1], f32, tag=f"rcp{c}", name=f"rcp{c}")
            nc.vector.reciprocal(rcp[:], sme[:])
            # A1T = (Ex * rcp) * WT   (= S^T o W^T)
            nc.vector.scalar_tensor_tensor(A1T[c][:], Ex[c][:], rcp[:, 0:1],
                                           WT[c][:], op0=AL.mult, op1=AL.mult)
            # diag: dS = sum_r (Ex*rcp)*I over the diagonal block
            dtmp = sg.tile([P, P], bf16, tag="dtmp")
            nc.vector.scalar_tensor_tensor(dtmp[:], Ex[c][:, c * P:(c + 1) * P],
                                           rcp[:, 0:1], I128b[:],
                                           op0=AL.mult, op1=AL.mult)
            nc.vector.tensor_reduce(dSv[c][:], dtmp[:], axis=AX.X, op=AL.add)

        # ---- dS row + broadcast tile (128, 512)
        dSrp = sml.tile([1, N], f32, tag="sml", name="dsrp")
        for c in range(CH):
            nc.tensor.transpose(dSrp[:, c * P:(c + 1) * P], dSv[c][:], I128f[:])
        dSrow = sg.tile([1, N], bf16, tag="dsrow")
        nc.scalar.copy(dSrow[:], dSrp[:])
        dSBp = sml.tile([P, N], f32, tag="sml", name="dsbp")
        nc.tensor.matmul(dSBp[:], ones1[:], dSrow[:], start=True, stop=True)
        nc.scalar.copy(dSB[:], dSBp[:])

        # ---- Tx0 in n-layout (all t at once)
        for k in range(CH):
            nc.vector.tensor_scalar(Tx0n[k][:], Xn[k][:], dSv[k][:, 0:1], None,
                                    op0=AL.mult)

        # =====================================================
        # Cheb + conv + LN, software-pipelined per pair
        # =====================================================
        Tx0P = {}
        TAp = {}
        Tx1T = {}
        ptA = {}
        Tx1n = {}
        TBp = {}
        Tx2T = {}
        TCp = {}
        XhP = {-1: zerot, NP: zerot}
        TDp = {}
        ZT = {}

        def e_tx0p(q):
            t = txp.tile([P, N], bf16, tag="tx0p", name=f"tx0p{q}")
            nc.gpsimd.tensor_tensor(t[:], Xw[q][:], dSB[:], op=AL.mult)
            Tx0P[q] = t

        def e_ta(q):
            p = big.tile([P, N], f32, tag="big", name=f"ta{q}")
            for k in range(CH):
                lhs = Tx0n[k][:, 2 * q * F:(2 * q + 2) * F]
                nc.tensor.matmul(p[:], lhs, A1T[k][:],
                                 start=(k == 0), stop=(k == CH - 1))
            TAp[q] = p

        def e_b(q):
            t = txp.tile([P, N], bf16, tag="tx1t", name=f"tx1t{q}")
            nc.vector.tensor_copy(t[:], TAp[q][:])
            Tx1T[q] = t

        def e_c(q):
            p = hlf.tile([P, N], bf16, tag="hlf", name=f"pta{q}")
            for k in range(CH):
                nc.tensor.transpose(p[:, k * P:(k + 1) * P],
                                    Tx1T[q][:, k * P:(k + 1) * P], I128b[:])
            ptA[q] = p

        def e_d(q):
            t = txp.tile([P, N], bf16, tag="tx1n", name=f"tx1n{q}")
            nc.scalar.copy(t[:], ptA[q][:])
            Tx1n[q] = t

        def e_e(q):
            p = big.tile([P, N], f32, tag="big", name=f"tb{q}")
            for k in range(CH):
                nc.tensor.matmul(p[:], Tx1n[q][:, k * P:(k + 1) * P], WT[k][:],
                                 start=(k == 0), stop=(k == CH - 1))
            TBp[q] = p

        def e_f(q):
            t = txp.tile([P, N], bf16, tag="tx2t", name=f"tx2t{q}")
            nc.vector.scalar_tensor_tensor(t[:], TBp[q][:], 2.0, Tx0P[q][:],
                                           op0=AL.mult, op1=AL.subtract)
            Tx2T[q] = t

        def e_g(q):
            p = big.tile([P, N], f32, tag="big", name=f"tc{q}")
            nc.tensor.matmul(p[:], WcP[0][:], Tx0P[q][:], start=True, stop=False)
            nc.tensor.matmul(p[:], WcP[1][:], Tx1T[q][:], start=False, stop=False)
            nc.tensor.matmul(p[:], WcP[2][:], Tx2T[q][:], start=False, stop=True)
            TCp[q] = p

        def e_h(q):
            t = xhp.tile([P, N], bf16, tag="xh", name=f"xh{q}")
            nc.scalar.activation(t[:], TCp[q][:], AF.Relu, bias=bch[:, 0:1],
                                 scale=1.0)
            XhP[q] = t

        def e_i(q):
            p = big.tile([P, N], f32, tag="big", name=f"td{q}")
            nc.tensor.matmul(p[:], Lprev[:], XhP[q - 1][:], start=True, stop=False)
            nc.tensor.matmul(p[:], Lmid[:], XhP[q][:], start=False, stop=False)
            nc.tensor.matmul(p[:], Lnext[:], XhP[q + 1][:], start=False, stop=False)
            nc.tensor.matmul(p[:], WrP[:], Xw[q][:], start=False, stop=True)
            TDp[q] = p

        def e_j(q):
            t = lnp.tile([P, N], bf16, tag="zt", name=f"zt{q}")
            nc.scalar.activation(t[:], TDp[q][:], AF.Relu, bias=btr[:, 0:1],
                                 scale=1.0)
            ZT[q] = t

        RSTD = {}
        NMR = {}

        def e_ln1(q):
            z = ZT[q]
            sq = lnp.tile([P, N], bf16, tag="sq", name=f"sq{q}")
            nc.vector.tensor_tensor(sq[:], z[:], z[:], op=AL.mult)
            # stats: rows 0:2 mean per v, rows 32:34 E[x^2] per v (B2 = 1/64)
            # (matmul out base partition must be 0/32/64)
            s12 = sml.tile([34, N], f32, tag="sml", name=f"s12{q}")
            nc.tensor.matmul(s12[0:2, :], B2[:], z[:], start=True, stop=True)
            nc.tensor.matmul(s12[32:34, :], B2[:], sq[:], start=True, stop=True)
            mu = lnp.tile([2, N], f32, tag="mu", name=f"mu{q}")
            nc.scalar.copy(mu[:], s12[0:2, :])
            mu2 = lnp.tile([2, N], f32, tag="mu2", name=f"mu2{q}")
            nc.vector.tensor_tensor(mu2[:], mu[:], mu[:], op=AL.mult)
            # var+eps = (msq + eps) - mu^2 in one stt
            var = lnp.tile([2, N], f32, tag="var", name=f"var{q}")
            nc.vector.scalar_tensor_tensor(var[:], s12[32:34, :], LN_EPS, mu2[:],
                                           op0=AL.add, op1=AL.subtract)
            # rstd = sqrt(1/(var+eps)); approx recip is ~18 bits, plenty
            rv = lnp.tile([2, N], f32, tag="rv", name=f"rv{q}")
            nc.vector.reciprocal_approx_fast(rv[:], var[:])
            rstd = lnp.tile([2, N], bf16, tag="rstd", name=f"rstd{q}")
            nc.scalar.activation(rstd[:], rv[:], AF.Sqrt)
            RSTD[q] = rstd
            # nmr = mu * rstd
            nmr = lnp.tile([2, N], bf16, tag="nmr", name=f"nmr{q}")
            nc.gpsimd.tensor_tensor(nmr[:], mu[:], rstd[:], op=AL.mult)
            NMR[q] = nmr

        def e_ln2(q):
            z = ZT[q]
            # broadcast to 128 partitions
            rBp = big.tile([P, N], f32, tag="big", name=f"rbp{q}")
            nc.tensor.matmul(rBp[:], B2T[:], RSTD[q][:], start=True, stop=True)
            nBp = big.tile([P, N], f32, tag="big", name=f"nbp{q}")
            nc.tensor.matmul(nBp[:], B2T[:], NMR[q][:], start=True, stop=True)
            # w = (z*rB)*gam - (nB*gam - bet)
            u = lnp.tile([P, N], bf16, tag="u", name=f"u{q}")
            nc.vector.tensor_tensor(u[:], z[:], rBp[:], op=AL.mult)
            nB2 = lnp.tile([P, N], bf16, tag="nb2", name=f"nb2{q}")
            nc.scalar.activation(nB2[:], nBp[:], AF.Identity,
                                 bias=nbetP[:, 0:1], scale=gamP[:, 0:1])
            w = lnp.tile([P, N], bf16, tag="w", name=f"w{q}")
            nc.vector.scalar_tensor_tensor(w[:], u[:], gamP[:, 0:1], nB2[:],
                                           op0=AL.mult, op1=AL.subtract)
            nc.sync.dma_start(out=ZoutD[q * P:(q + 1) * P, :], in_=w[:])

        # pipeline drive: iteration i handles ln2(i-3), cheb stages for
        # (i-1)/(i), conv+ln1 for (i-2). Each engine's queue is emitted in
        # readiness order so no engine head-of-line-blocks on a later stage.
        def live(q):
            return 0 <= q < NP

        for i in range(NP + 3):
            if live(i - 3):
                e_ln2(i - 3)
            if live(i - 1):
                e_c(i - 1)
            if live(i):
                if i == 0:
                    e_tx0p(0)
                e_ta(i)
            if live(i - 1):
                e_d(i - 1)
                e_e(i - 1)
                e_f(i - 1)
                e_g(i - 1)
                e_h(i - 1)
            if live(i):
                e_b(i)
                if live(i + 1):
                    e_tx0p(i + 1)
            if live(i - 2):
                e_i(i - 2)
                e_j(i - 2)
                e_ln1(i - 2)

    nc.compile()
    return nc


def _host_prep(inputs):
    import ml_dtypes
    bf = ml_dtypes.bfloat16

    X = np.asarray(inputs['X'], np.float32)
    edge_index = np.asarray(inputs['edge_index'])
    U1 = np.asarray(inputs['U1'], np.float32)
    U2 = np.asarray(inputs['U2'], np.float32)
    U3 = np.asarray(inputs['U3'], np.float32)
    be = np.asarray(inputs['be'], np.float32)
    Ve = np.asarray(inputs['Ve'], np.float32)
    Ws1 = np.asarray(inputs['Ws1'], np.float32)
    Ws2 = np.asarray(inputs['Ws2'], np.float32)
    Ws3 = np.asarray(inputs['Ws3'], np.float32)
    bs = np.asarray(inputs['bs'], np.float32)
    Vs = np.asarray(inputs['Vs'], np.float32)
    W_cheb = np.asarray(inputs['W_cheb'], np.float32)
    b_cheb = np.asarray(inputs['b_cheb'], np.float32)
    Wt = np.asarray(inputs['Wt'], np.float32)
    bt = np.asarray(inputs['bt'], np.float32)
    Wr = np.asarray(inputs['Wr'], np.float32)
    br = np.asarray(inputs['br'], np.float32)
    gamma = np.asarray(inputs['gamma'], np.float32)
    beta = np.asarray(inputs['beta'], np.float32)

    # dense symmetric-norm matrix (self-loop +I/-I terms cancel)
    row, col = edge_index[0].astype(np.int64), edge_index[1].astype(np.int64)
    deg = np.zeros(N, np.float32)
    np.add.at(deg, row, 1.0)
    dis = np.where(deg > 0, 1.0 / np.sqrt(np.maximum(deg, 1.0)), 0.0).astype(np.float32)
    wn = -dis[row] * dis[col]
    W = np.zeros((N, N), np.float32)
    np.add.at(W, (row, col), wn)

    # conv block matrices: L[(v,fi),(u,fo)] = Wt[fo,fi,0,dt]
    WtT = [np.ascontiguousarray(Wt[:, :, 0, d].T) for d in range(3)]  # (fi,fo)
    Z64 = np.zeros((F, F), np.float32)
    Lmid = np.block([[WtT[1], WtT[0]], [WtT[2], WtT[1]]]).astype(bf)
    Lprev = np.block([[Z64, Z64], [WtT[0], Z64]]).astype(bf)
    Lnext = np.block([[Z64, WtT[2]], [Z64, Z64]]).astype(bf)
    WrT = np.ascontiguousarray(Wr[:, :, 0, 0].T)
    WrP = np.block([[WrT, Z64], [Z64, WrT]]).astype(bf)
    WcP = np.stack([np.block([[W_cheb[k], Z64], [Z64, W_cheb[k]]]) for k in range(3)]
                   ).astype(bf)

    Wpk = np.stack([WcP[0], WcP[1], WcP[2], Lprev, Lmid, Lnext, WrP])

    Pf = np.zeros((P, PFW), np.float32)
    Pf[:, 0] = np.tile(gamma, 2)
    Pf[:, 1] = np.tile(-beta, 2)
    Pf[:, 2] = np.tile(b_cheb, 2)
    Pf[:, 3] = np.tile(bt + br, 2)
    Pf[:, 4:132] = np.eye(P, dtype=np.float32)

    VsTh = 0.5 * np.ascontiguousarray(Vs.T)
    vch = VsTh.sum(axis=0)                 # 0.5*colsum(Vs^T) sigmoid-fold row
    shared = {
        'bsh': (0.5 * bs[0]).astype(bf),
        'VsT': VsTh.astype(bf),
        'WT': np.ascontiguousarray(W.T).astype(bf),
        'Wpk': Wpk,
    }

    in_maps = []
    for core in range(8):
        b, h = core // 2, core % 2
        tmap = list(range(16)) if h == 0 else list(range(6, 16)) + list(range(6))
        Xp = X[b][:, :, tmap]                              # (N, F, 16)
        Xn = np.ascontiguousarray(Xp.transpose(0, 2, 1).reshape(N, T * F)).astype(bf)
        Xw = np.ascontiguousarray(Xp.transpose(2, 1, 0).reshape(8, P, N)).astype(bf)
        UW = np.zeros((8, P, 48), np.float32)
        for tp in range(16):
            s, v = tp // 2, tp % 2
            UW[s, 64 * v:64 * v + 64, tp] = Ws3
            UW[s, 64 * v:64 * v + 64, 32 + tp] = U3
        Pb = np.zeros((P, PBW), np.float32)
        Pb[:, 0:4] = U1.reshape(4, P).T
        Pb[:, 4:20] = np.vstack([Ws2, Ws2])
        VeTh = 0.5 * Ve[np.ix_(tmap, tmap)].T
        Pb[0:16, 20:36] = VeTh
        Pb[0:16, 36] = Ws1[tmap]
        Pb[0, 37:165] = 1.0
        Pb[:, 165:293] = np.eye(P, dtype=np.float32)
        Pb[0:64, 293:805] = U2
        Pb[0, 805:1061] = np.eye(T, dtype=np.float32).reshape(-1)
        # B2: (128, 2) block indicator * 1/64 for per-v mean over f
        Pb[0:64, 1061] = 1.0 / 64
        Pb[64:128, 1062] = 1.0 / 64
        # B2T: (2, 128) block indicator for broadcast back
        Pb[0, 1063:1127] = 1.0
        Pb[1, 1127:1191] = 1.0
        # sigmoid-fold rank-1 rows
        Pb[0, 1191:1207] = VeTh.sum(axis=0)
        Pb[0, 1207:1719] = vch
        Pfc = Pf.copy()
        Pfc[0:16, 132:148] = 0.5 * be[0][np.ix_(tmap, tmap)]
        m = dict(shared)
        m.update({
            'Xn': Xn, 'Xw': Xw, 'UW': UW.astype(bf),
            'Pb': Pb.astype(bf), 'Pf': Pfc,
        })
        in_maps.append(m)
    return in_maps


def kernel(**inputs):
    import sys
    if '/opt/trn_rl_repo' not in sys.path:
        sys.path.insert(0, '/opt/trn_rl_repo')
    from concourse.bass_utils import run_bass_kernel_spmd

    if 'nc' not in _CACHE:
        _CACHE['nc'] = _build_program()
    nc = _CACHE['nc']

    in_maps = _host_prep(inputs)
    res = run_bass_kernel_spmd(nc, in_maps, list(range(8)))
    out = np.zeros((B, N, F, T), np.float32)
    for core in range(8):
        b, h = core // 2, core % 2
        Z = np.asarray(res.results[core]['Zout']).astype(np.float32)
        # rows q*128 + v*64 + f, cols n  ->  (n, f, slot=2q+v)
        Zs = Z.reshape(NP, 2, F, N).transpose(3, 2, 0, 1).reshape(N, F, NSLOT)
        wstart = 0 if h == 0 else 6
        jlo = 0 if h == 0 else 2
        out[b, :, :, wstart + jlo:wstart + jlo + 8] = Zs[:, :, jlo:jlo + 8]
    return out


# revision 25
# speedup vs baseline: 1.4698x; 1.3600x over previous
"""ASTGCN block Trainium2 kernel (v2).

Strategy: 8 cores; core c handles batch b = c//2, time-half h = c%2 (8 output
timesteps each, data-parallel over B and T). Attention (temporal Et, spatial
S) is per-b and replicated on the 2 cores sharing a b. The sparse graph
propagation is reformulated as dense (N,N) matmuls: the edge-scatter of the
symmetric norm is accumulated host-side into a dense W (the +I/-I self-loop
terms cancel), so  prop1(h) = (W*S) @ h  and  prop2(h) = W @ h.

v2 changes vs baseline:
- Input DMAs ordered by first use (Pb/Pf/Xn first) and X tensors split in
  halves so attention matmuls start ~5us in instead of after all loads.
- Single activation-table regime: sigmoid via tanh (0.5*tanh(x/2)+0.5, in
  the exp table) and LN rstd via exp(-0.5*ln(var+eps)); only one table
  switch in the whole program (exp_and_others -> natural_log_exp...).
- LayerNorm runs in pair layout: per-pair stats via ones-block matmuls on
  PE (reduce over the f partition rows), rstd/-mu*rstd broadcast back with
  block matmuls; no transposes of the conv output at all.
- Output stored in pair layout as bf16; host does the final (f,n) -> (n,f)
  transpose and fp32 upcast.
- cheb -> conv -> LN -> store software-pipelined across the 5 timestep
  pairs to keep PE dense (p-state) and overlap store DMAs with compute.

Per-core time axis is PERMUTED so the program is identical SPMD: slot t' maps
to global t via tmap (identity for h=0, rotated by 6 for h=1); all
t-dependent weights (be, Ve, Ws1, UW) are permuted host-side to match.
"""

import numpy as np

B, N, F, T = 4, 512, 64, 16
P = 128
CH = N // P            # 4 n-chunks
NSLOT = 10             # cheb window timesteps per core (5 pairs)
NP = NSLOT // 2        # 5 pairs
LN_EPS = 1e-5

PBW = 1719             # packed bf16 constant width
PFW = 148              # packed f32 constant width

_CACHE = {}


def _build_program():
    import sys
    if '/opt/trn_rl_repo' not in sys.path:
        sys.path.insert(0, '/opt/trn_rl_repo')
    from contextlib import ExitStack
    import concourse.bass as bass
    import concourse.tile as tile
    from concourse import bacc, mybir

    dt = mybir.dt
    AL = mybir.AluOpType
    AF = mybir.ActivationFunctionType
    AX = mybir.AxisListType
    f32 = dt.float32
    bf16 = dt.bfloat16

    nc = bacc.Bacc("TRN2", target_bir_lowering=False, debug=False, num_devices=1)

    def din(name, shape, d=bf16):
        return nc.dram_tensor(name, list(shape), d, kind="ExternalInput").ap()

    XnD   = din("Xn", (N, T * F))
    XwD   = din("Xw", (8, P, N))
    UWD   = din("UW", (8, P, 48))
    bshD  = din("bsh", (N, N))          # 0.5 * bs
    VsTD  = din("VsT", (N, N))
    WTD   = din("WT", (N, N))
    WpkD  = din("Wpk", (7, P, P))
    PbD   = din("Pb", (P, PBW))
    PfD   = din("Pf", (P, PFW), f32)
    ZoutD = nc.dram_tensor("Zout", [NP * P, N], bf16, kind="ExternalOutput").ap()

    with tile.TileContext(nc) as tc, ExitStack() as ctx:
        sg = ctx.enter_context(tc.tile_pool(name="sg", bufs=1))
        big = ctx.enter_context(tc.tile_pool(name="big", bufs=5, space="PSUM"))
        sml = ctx.enter_context(tc.tile_pool(name="sml", bufs=2, space="PSUM"))
        hlf = ctx.enter_context(tc.tile_pool(name="hlf", bufs=1, space="PSUM"))
        xhp = ctx.enter_context(tc.tile_pool(name="xhp", bufs=7))
        txp = ctx.enter_context(tc.tile_pool(name="txp", bufs=3))
        lnp = ctx.enter_context(tc.tile_pool(name="lnp", bufs=3))

        # ------------- input DMAs, ordered by first use -------------
        Pb = sg.tile([P, PBW], bf16, tag="pb")
        nc.sync.dma_start(out=Pb[:], in_=PbD)
        Pf = sg.tile([P, PFW], f32, tag="pf")
        nc.sync.dma_start(out=Pf[:], in_=PfD)
        XnA = sg.tile([P, 2, T * F], bf16, tag="xna")
        XnB = sg.tile([P, 2, T * F], bf16, tag="xnb")
        XnDr = XnD.rearrange("(k p) t -> p k t", k=CH)
        nc.sync.dma_start(out=XnA[:], in_=XnDr[:, 0:2, :])
        nc.sync.dma_start(out=XnB[:], in_=XnDr[:, 2:4, :])
        UWAll = sg.tile([P, 8, 48], bf16, tag="uwall")
        nc.sync.dma_start(out=UWAll[:], in_=UWD.rearrange("s p n -> p s n"))
        XwA = sg.tile([P, 4, N], bf16, tag="xwa")
        XwB = sg.tile([P, 4, N], bf16, tag="xwb")
        XwDr = XwD.rearrange("s p n -> p s n")
        nc.sync.dma_start(out=XwA[:], in_=XwDr[:, 0:4, :])
        nc.sync.dma_start(out=XwB[:], in_=XwDr[:, 4:8, :])
        bsAll = sg.tile([P, CH, N], bf16, tag="bsall")
        nc.sync.dma_start(out=bsAll[:], in_=bshD.rearrange("(k p) n -> p k n", k=CH))
        VsTAll = sg.tile([P, CH, N], bf16, tag="vstall")
        nc.sync.dma_start(out=VsTAll[:], in_=VsTD.rearrange("(k p) n -> p k n", k=CH))
        WTAll = sg.tile([P, CH, N], bf16, tag="wtall")
        nc.sync.dma_start(out=WTAll[:], in_=WTD.rearrange("(k p) n -> p k n", k=CH))
        Wpk = sg.tile([P, 7, P], bf16, tag="wpk")
        nc.sync.dma_start(out=Wpk[:], in_=WpkD.rearrange("w p c -> p w c"))

        Xn = [XnA[:, 0, :], XnA[:, 1, :], XnB[:, 0, :], XnB[:, 1, :]]
        Xw = [XwA[:, s, :] for s in range(4)] + [XwB[:, s, :] for s in range(4)]
        UW = [UWAll[:, s, :] for s in range(8)]
        bsh = [bsAll[:, k, :] for k in range(CH)]
        VsT = [VsTAll[:, k, :] for k in range(CH)]
        WT = [WTAll[:, k, :] for k in range(CH)]
        WcP = [Wpk[:, k, :] for k in range(3)]
        Lprev, Lmid, Lnext, WrP = (Wpk[:, 3, :], Wpk[:, 4, :], Wpk[:, 5, :],
                                   Wpk[:, 6, :])
        # packed bf16 layout
        U1r = Pb[:, 0:4]
        Ws2d = Pb[:, 4:20]
        VeT = Pb[0:16, 20:36]
        Ws1 = Pb[0:16, 36:37]
        ones1 = Pb[0:1, 37:165]
        I128b = Pb[:, 165:293]
        U2 = Pb[0:64, 293:805]
        I16r = Pb[0:1, 805:1061]     # I16 rows flattened: e_t = [0:1, 16t:16t+16]
        B2 = Pb[:, 1061:1063]        # (128,2) block col-indicator * 1/64
        B2T = Pb[0:2, 1063:1191]     # (2,128) block row-indicator
        hcVe = Pb[0:1, 1191:1207]    # 0.5*colsum(VeT')  [sigmoid-fold row]
        vch = Pb[0:1, 1207:1719]     # 0.5*colsum(VsT')  [sigmoid-fold row]
        # packed f32 layout
        gamP = Pf[:, 0:1]
        nbetP = Pf[:, 1:2]           # -beta
        bch = Pf[:, 2:3]
        btr = Pf[:, 3:4]
        I128f = Pf[:, 4:132]
        bePh = Pf[0:16, 132:148]     # 0.5 * be (permuted)

        zerot = sg.tile([P, N], bf16, tag="zerot")
        nc.vector.memset(zerot[:], 0.0)
        epsP = sg.tile([P, 1], f32, tag="epsP")
        nc.vector.memset(epsP[:], LN_EPS)

        # persistent sbuf intermediates
        G = [sg.tile([P, N], bf16, tag=f"g{k}", name=f"g{k}") for k in range(CH)]
        Ex = [sg.tile([P, N], bf16, tag=f"ex{k}", name=f"ex{k}") for k in range(CH)]
        A1T = [sg.tile([P, N], bf16, tag=f"a1t{k}", name=f"a1t{k}") for k in range(CH)]
        dSv = [sg.tile([P, 1], f32, tag=f"dsv{k}", name=f"dsv{k}") for k in range(CH)]
        Tx0n = [sg.tile([P, T * F], bf16, tag=f"tx0n{k}", name=f"tx0n{k}")
                for k in range(CH)]
        dSB = sg.tile([P, N], bf16, tag="dsb")

        # =====================================================
        # Attention phase
        # =====================================================
        # ---- lhs0[(t,f)] = sum_n U1[n] X[n,(t,f)]  -> (1,1024)
        L0a = sml.tile([1, 512], f32, tag="sml", name="l0a")
        for k in range(CH):
            nc.tensor.matmul(L0a[:], U1r[:, k:k + 1], Xn[k][:, 0:512],
                             start=(k == 0), stop=(k == CH - 1))
        L0b = sml.tile([1, 512], f32, tag="sml", name="l0b")
        for k in range(CH):
            nc.tensor.matmul(L0b[:], U1r[:, k:k + 1], Xn[k][:, 512:1024],
                             start=(k == 0), stop=(k == CH - 1))
        lhs0row = sg.tile([1, T * F], bf16, tag="lhs0row")
        nc.vector.tensor_copy(lhs0row[:, 0:512], L0a[:])
        nc.vector.tensor_copy(lhs0row[:, 512:1024], L0b[:])
        # reshape to (64,16) via 16 rank-1 matmuls against identity rows
        l0Fp = sml.tile([F, T], f32, tag="sml", name="l0fp")
        for t in range(T):
            nc.tensor.matmul(l0Fp[:], lhs0row[0:1, 64 * t:64 * t + 64],
                             I16r[0:1, 16 * t:16 * t + 16],
                             start=(t == 0), stop=(t == T - 1))
        lhs0F = sg.tile([F, T], bf16, tag="lhs0f")
        nc.vector.tensor_copy(lhs0F[:], l0Fp[:])

        # ---- lhs2T chunks (n,16) = U2[:,chunk].T @ lhs0F
        lhs2T = []
        for k in range(CH):
            pt = sml.tile([P, T], f32, tag="sml", name="l2t")
            nc.tensor.matmul(pt[:], U2[:, k * P:(k + 1) * P], lhs0F[:],
                             start=True, stop=True)
            st = sg.tile([P, T], bf16, tag=f"l2ts{k}", name=f"l2ts{k}")
            nc.vector.tensor_copy(st[:], pt[:])
            lhs2T.append(st)

        # ---- R48: rows 0:16 rhs3T (Ws3), rows 32:48 rhs_tT (U3)
        R48p = sml.tile([48, N], f32, tag="sml", name="r48")
        for s in range(8):
            nc.tensor.matmul(R48p[:], UW[s][:, :], Xw[s][:, :],
                             start=(s == 0), stop=(s == 7))
        R48 = sg.tile([48, N], bf16, tag="r48s")
        nc.scalar.copy(R48[:], R48p[:])

        # ---- rhs_tn chunks: transpose R48[32:48]
        rhs_tn = []
        for k in range(CH):
            pt = hlf.tile([P, T], bf16, tag="hlf", name="rtn")
            nc.tensor.transpose(pt[:], R48[32:48, k * P:(k + 1) * P],
                                I128b[32:48, 32:48])
            st = sg.tile([P, T], bf16, tag=f"rtns{k}", name=f"rtns{k}")
            nc.vector.tensor_copy(st[:], pt[:])
            rhs_tn.append(st)

        # ---- P0 (16,16) = lhs_t @ rhs_t ; sigmoid via tanh:
        # sig = 0.5*tanh(0.5*(P0+be)) + 0.5
        P0p = sml.tile([T, T], f32, tag="sml", name="p0")
        for k in range(CH):
            nc.tensor.matmul(P0p[:], lhs2T[k][:], rhs_tn[k][:],
                             start=(k == 0), stop=(k == CH - 1))
        # sigmoid fold: sig = 0.5*tanh(0.5*(P0+be)) + 0.5; the affine is folded
        # into the E1T matmul (VeT is 0.5-scaled host-side, hcVe rank-1 term)
        sig = sg.tile([T, T], bf16, tag="sig")
        nc.vector.scalar_tensor_tensor(sig[:], P0p[:], 0.5, bePh[:],
                                       op0=AL.mult, op1=AL.add)
        nc.scalar.activation(sig[:], sig[:], AF.Tanh)

        # ---- E1^T = sigmoid^T @ Ve^T ; softmax over free dim
        E1Tp = sml.tile([T, T], f32, tag="sml", name="e1t")
        nc.tensor.matmul(E1Tp[:], sig[:], VeT[:], start=True, stop=False)
        nc.tensor.matmul(E1Tp[:], ones1[0:1, 0:16], hcVe[:],
                         start=False, stop=True)
        E1Ts = sg.tile([T, T], bf16, tag="e1ts")
        nc.vector.tensor_copy(E1Ts[:], E1Tp[:])
        # values are O(1e-1): skip the max-subtraction for softmax
        sume = sg.tile([T, 1], f32, tag="sume")
        EtT = sg.tile([T, T], bf16, tag="ett")
        nc.scalar.activation(EtT[:], E1Ts[:], AF.Exp,
                             scale=1.0, accum_out=sume[:, 0:1])
        rse = sg.tile([T, 1], f32, tag="rse")
        nc.vector.reciprocal(rse[:], sume[:])
        nc.vector.tensor_scalar(EtT[:], EtT[:], rse[:, 0:1], None, op0=AL.mult)
        Etp = hlf.tile([T, T], bf16, tag="hlf", name="etp")
        nc.tensor.transpose(Etp[:], EtT[:], I128b[0:16, 0:16])
        Et = sg.tile([T, T], bf16, tag="et")
        nc.vector.tensor_copy(Et[:], Etp[:])

        # ---- w1e row (1,16) = Ws1.T @ EtT ; broadcast to (128,16)
        w1p = sml.tile([1, T], f32, tag="sml", name="w1p")
        nc.tensor.matmul(w1p[:], Ws1[:], EtT[:], start=True, stop=True)
        w1row = sg.tile([1, T], bf16, tag="w1row")
        nc.scalar.copy(w1row[:], w1p[:])
        w1Bp = sml.tile([P, T], f32, tag="sml", name="w1bp")
        nc.tensor.matmul(w1Bp[:], ones1[:], w1row[:], start=True, stop=True)
        w1B = sg.tile([P, T], bf16, tag="w1b")
        nc.vector.tensor_copy(w1B[:], w1Bp[:])

        # ---- w1Bpair[p=(v,f), s] = w1e[2s+v]
        w1Bp2 = sg.tile([P, 8], bf16, tag="w1bp2")
        nc.vector.tensor_copy(w1Bp2[0:64, :], w1B[0:64, 0:T:2])
        nc.vector.tensor_copy(w1Bp2[64:128, :], w1B[64:128, 1:T:2])
        # ---- Ws2wP[p=(v,f), s, t] = Ws2d[p,t] * w1e[2s+v]   (128, 8, 16)
        Ws2w = sg.tile([P, 8, T], bf16, tag="ws2w")
        nc.vector.tensor_tensor(
            Ws2w[:],
            Ws2d[:].unsqueeze(1).broadcast_to((P, 8, T)),
            w1Bp2[:].unsqueeze(2).broadcast_to((P, 8, T)),
            op=AL.mult)

        # ---- lhs_sT (16, 512) = sum_t1 (Ws2*w1e[t1]).T @ X^T[t1]
        lsTp = sml.tile([T, N], f32, tag="sml", name="lst")
        for s in range(8):
            nc.tensor.matmul(lsTp[:], Ws2w[:, s, :], Xw[s][:, :],
                             start=(s == 0), stop=(s == 7))
        lsT = sg.tile([T, N], bf16, tag="lsts")
        nc.scalar.copy(lsT[:], lsTp[:])

        # ---- rhs_s (16, 512) = Et-weighted rhs3
        rsp = sml.tile([T, N], f32, tag="sml", name="rsp")
        nc.tensor.matmul(rsp[:], Et[:], R48[0:16, :], start=True, stop=True)
        rss = sg.tile([T, N], bf16, tag="rss")
        nc.scalar.copy(rss[:], rsp[:])

        # ---- P chunks; G holds tanh(0.5*P + bsh); the sigmoid affine is
        # folded into M1T (VsT 0.5-scaled host-side + vch rank-1 term)
        for k in range(CH):
            Pp = big.tile([P, N], f32, tag="big", name="pp")
            nc.tensor.matmul(Pp[:], lsT[:, k * P:(k + 1) * P], rss[:],
                             start=True, stop=True)
            nc.vector.scalar_tensor_tensor(G[k][:], Pp[:], 0.5, bsh[k][:],
                                           op0=AL.mult, op1=AL.add)
            nc.scalar.activation(G[k][:], G[k][:], AF.Tanh)

        # ---- M1T chunks (c-part, r) + masked softmax -> A1T, dS
        for c in range(CH):
            Mp = big.tile([P, N], f32, tag="big", name="mp")
            for k in range(CH):
                nc.tensor.matmul(Mp[:], G[k][:, c * P:(c + 1) * P], VsT[k][:],
                                 start=(k == 0), stop=False)
            nc.tensor.matmul(Mp[:], ones1[0:1, 0:128], vch[:],
                             start=False, stop=True)
            sme = sg.tile([P, 1], f32, tag=f"sme{c}", name=f"sme{c}")
            nc.scalar.activation(Ex[c][:], Mp[:], AF.Exp,
                                 scale=1.0, accum_out=sme[:, 0:1])
            rcp = sg.tile([P, 1], f32, tag=f"rcp{c}", name=f"rcp{c}")
            nc.vector.reciprocal(rcp[:], sme[:])
            # A1T = (Ex * rcp) * WT   (= S^T o W^T)
            nc.vector.scalar_tensor_tensor(A1T[c][:], Ex[c][:], rcp[:, 0:1],
                                           WT[c][:], op0=AL.mult, op1=AL.mult)
            # diag: dS = sum_r (Ex*rcp)*I over the diagonal block
            dtmp = sg.tile([P, P], bf16, tag="dtmp")
            nc.vector.scalar_tensor_tensor(dtmp[:], Ex[c][:, c * P:(c + 1) * P],
                                           rcp[:, 0:1], I128b[:],
                                           op0=AL.mult, op1=AL.mult)
            nc.vector.tensor_reduce(dSv[c][:], dtmp[:], axis=AX.X, op=AL.add)

        # ---- dS row + broadcast tile (128, 512)
        dSrp = sml.tile([1, N], f32, tag="sml", name="dsrp")
        for c in range(CH):
            nc.tensor.transpose(dSrp[:, c * P:(c + 1) * P], dSv[c][:], I128f[:])
        dSrow = sg.tile([1, N], bf16, tag="dsrow")
        nc.scalar.copy(dSrow[:], dSrp[:])
        dSBp = sml.tile([P, N], f32, tag="sml", name="dsbp")
        nc.tensor.matmul(dSBp[:], ones1[:], dSrow[:], start=True, stop=True)
        nc.scalar.copy(dSB[:], dSBp[:])

        # ---- Tx0 in n-layout (all t at once)
        for k in range(CH):
            nc.vector.tensor_scalar(Tx0n[k][:], Xn[k][:], dSv[k][:, 0:1], None,
                                    op0=AL.mult)

        # =====================================================
        # Cheb + conv + LN, software-pipelined per pair
        # =====================================================
        Tx0P = {}
        TAp = {}
        Tx1T = {}
        ptA = {}
        Tx1n = {}
        TBp = {}
        Tx2T = {}
        TCp = {}
        XhP = {-1: zerot, NP: zerot}
        TDp = {}
        ZT = {}

        def e_tx0p(q):
            t = txp.tile([P, N], bf16, tag="tx0p", name=f"tx0p{q}")
            nc.gpsimd.tensor_tensor(t[:], Xw[q][:], dSB[:], op=AL.mult)
            Tx0P[q] = t

        def e_ta(q):
            p = big.tile([P, N], f32, tag="big", name=f"ta{q}")
            for k in range(CH):
                lhs = Tx0n[k][:, 2 * q * F:(2 * q + 2) * F]
                nc.tensor.matmul(p[:], lhs, A1T[k][:],
                                 start=(k == 0), stop=(k == CH - 1))
            TAp[q] = p

        def e_b(q):
            t = txp.tile([P, N], bf16, tag="tx1t", name=f"tx1t{q}")
            nc.vector.tensor_copy(t[:], TAp[q][:])
            Tx1T[q] = t

        def e_c(q):
            p = hlf.tile([P, N], bf16, tag="hlf", name=f"pta{q}")
            for k in range(CH):
                nc.tensor.transpose(p[:, k * P:(k + 1) * P],
                                    Tx1T[q][:, k * P:(k + 1) * P], I128b[:])
            ptA[q] = p

        def e_d(q):
            t = txp.tile([P, N], bf16, tag="tx1n", name=f"tx1n{q}")
            nc.scalar.copy(t[:], ptA[q][:])
            Tx1n[q] = t

        def e_e(q):
            p = big.tile([P, N], f32, tag="big", name=f"tb{q}")
            for k in range(CH):
                nc.tensor.matmul(p[:], Tx1n[q][:, k * P:(k + 1) * P], WT[k][:],
                                 start=(k == 0), stop=(k == CH - 1))
            TBp[q] = p

        def e_f(q):
            t = txp.tile([P, N], bf16, tag="tx2t", name=f"tx2t{q}")
            nc.vector.scalar_tensor_tensor(t[:], TBp[q][:], 2.0, Tx0P[q][:],
                                           op0=AL.mult, op1=AL.subtract)
            Tx2T[q] = t

        def e_g(q):
            p = big.tile([P, N], f32, tag="big", name=f"tc{q}")
            nc.tensor.matmul(p[:], WcP[0][:], Tx0P[q][:], start=True, stop=False)
            nc.tensor.matmul(p[:], WcP[1][:], Tx1T[q][:], start=False, stop=False)
            nc.tensor.matmul(p[:], WcP[2][:], Tx2T[q][:], start=False, stop=True)
            TCp[q] = p

        def e_h(q):
            t = xhp.tile([P, N], bf16, tag="xh", name=f"xh{q}")
            nc.scalar.activation(t[:], TCp[q][:], AF.Relu, bias=bch[:, 0:1],
                                 scale=1.0)
            XhP[q] = t

        def e_i(q):
            p = big.tile([P, N], f32, tag="big", name=f"td{q}")
            nc.tensor.matmul(p[:], Lprev[:], XhP[q - 1][:], start=True, stop=False)
            nc.tensor.matmul(p[:], Lmid[:], XhP[q][:], start=False, stop=False)
            nc.tensor.matmul(p[:], Lnext[:], XhP[q + 1][:], start=False, stop=False)
            nc.tensor.matmul(p[:], WrP[:], Xw[q][:], start=False, stop=True)
            TDp[q] = p

        def e_j(q):
            t = lnp.tile([P, N], bf16, tag="zt", name=f"zt{q}")
            nc.scalar.activation(t[:], TDp[q][:], AF.Relu, bias=btr[:, 0:1],
                                 scale=1.0)
            ZT[q] = t

        RSTD = {}
        NMR = {}

        def e_ln1(q):
            z = ZT[q]
            sq = lnp.tile([P, N], bf16, tag="sq", name=f"sq{q}")
            nc.vector.tensor_tensor(sq[:], z[:], z[:], op=AL.mult)
            # stats: rows 0:2 mean per v, rows 32:34 E[x^2] per v (B2 = 1/64)
            # (matmul out base partition must be 0/32/64)
            s12 = sml.tile([34, N], f32, tag="sml", name=f"s12{q}")
            nc.tensor.matmul(s12[0:2, :], B2[:], z[:], start=True, stop=True)
            nc.tensor.matmul(s12[32:34, :], B2[:], sq[:], start=True, stop=True)
            mu = lnp.tile([2, N], f32, tag="mu", name=f"mu{q}")
            nc.scalar.copy(mu[:], s12[0:2, :])
            mu2 = lnp.tile([2, N], f32, tag="mu2", name=f"mu2{q}")
            nc.vector.tensor_tensor(mu2[:], mu[:], mu[:], op=AL.mult)
            # var+eps = (msq + eps) - mu^2 in one stt
            var = lnp.tile([2, N], f32, tag="var", name=f"var{q}")
            nc.vector.scalar_tensor_tensor(var[:], s12[32:34, :], LN_EPS, mu2[:],
                                           op0=AL.add, op1=AL.subtract)
            # rstd = sqrt(1/(var+eps)); approx recip is ~18 bits, plenty
            rv = lnp.tile([2, N], f32, tag="rv", name=f"rv{q}")
            nc.vector.reciprocal_approx_fast(rv[:], var[:])
            rstd = lnp.tile([2, N], bf16, tag="rstd", name=f"rstd{q}")
            nc.scalar.activation(rstd[:], rv[:], AF.Sqrt)
            RSTD[q] = rstd
            # nmr = mu * rstd
            nmr = lnp.tile([2, N], bf16, tag="nmr", name=f"nmr{q}")
            nc.gpsimd.tensor_tensor(nmr[:], mu[:], rstd[:], op=AL.mult)
            NMR[q] = nmr

        def e_ln2(q):
            z = ZT[q]
            # broadcast to 128 partitions
            rBp = big.tile([P, N], f32, tag="big", name=f"rbp{q}")
            nc.tensor.matmul(rBp[:], B2T[:], RSTD[q][:], start=True, stop=True)
            nBp = big.tile([P, N], f32, tag="big", name=f"nbp{q}")
            nc.tensor.matmul(nBp[:], B2T[:], NMR[q][:], start=True, stop=True)
            # w = (z*rB)*gam - (nB*gam - bet)
            u = lnp.tile([P, N], bf16, tag="u", name=f"u{q}")
            nc.vector.tensor_tensor(u[:], z[:], rBp[:], op=AL.mult)
            nB2 = lnp.tile([P, N], bf16, tag="nb2", name=f"nb2{q}")
            nc.scalar.activation(nB2[:], nBp[:], AF.Identity,
                                 bias=nbetP[:, 0:1], scale=gamP[:, 0:1])
            w = lnp.tile([P, N], bf16, tag="w", name=f"w{q}")
            nc.vector.scalar_tensor_tensor(w[:], u[:], gamP[:, 0:1], nB2[:],
                                           op0=AL.mult, op1=AL.subtract)
            nc.sync.dma_start(out=ZoutD[q * P:(q + 1) * P, :], in_=w[:])

        # pipeline drive: iteration i handles ln2(i-4), cheb stages for
        # (i-1)/(i), conv+ln1 for (i-2). Each engine's queue is emitted in
        # readiness order so no engine head-of-line-blocks on a later stage.
        def live(q):
            return 0 <= q < NP

        for i in range(NP + 4):
            if live(i - 4):
                e_ln2(i - 4)
            if live(i - 1):
                e_c(i - 1)
            if live(i):
                if i == 0:
                    e_tx0p(0)
                e_ta(i)
            if live(i - 1):
                e_d(i - 1)
                e_e(i - 1)
                e_f(i - 1)
                e_g(i - 1)
                e_h(i - 1)
            if live(i):
                e_b(i)
                if live(i + 1):
                    e_tx0p(i + 1)
            if live(i - 2):
                e_i(i - 2)
                e_j(i - 2)
                e_ln1(i - 2)

    nc.compile()
    return nc


def _host_prep(inputs):
    import ml_dtypes
    bf = ml_dtypes.bfloat16

    X = np.asarray(inputs['X'], np.float32)
    edge_index = np.asarray(inputs['edge_index'])
    U1 = np.asarray(inputs['U1'], np.float32)
    U2 = np.asarray(inputs['U2'], np.float32)
    U3 = np.asarray(inputs['U3'], np.float32)
    be = np.asarray(inputs['be'], np.float32)
    Ve = np.asarray(inputs['Ve'], np.float32)
    Ws1 = np.asarray(inputs['Ws1'], np.float32)
    Ws2 = np.asarray(inputs['Ws2'], np.float32)
    Ws3 = np.asarray(inputs['Ws3'], np.float32)
    bs = np.asarray(inputs['bs'], np.float32)
    Vs = np.asarray(inputs['Vs'], np.float32)
    W_cheb = np.asarray(inputs['W_cheb'], np.float32)
    b_cheb = np.asarray(inputs['b_cheb'], np.float32)
    Wt = np.asarray(inputs['Wt'], np.float32)
    bt = np.asarray(inputs['bt'], np.float32)
    Wr = np.asarray(inputs['Wr'], np.float32)
    br = np.asarray(inputs['br'], np.float32)
    gamma = np.asarray(inputs['gamma'], np.float32)
    beta = np.asarray(inputs['beta'], np.float32)

    # dense symmetric-norm matrix (self-loop +I/-I terms cancel)
    row, col = edge_index[0].astype(np.int64), edge_index[1].astype(np.int64)
    deg = np.zeros(N, np.float32)
    np.add.at(deg, row, 1.0)
    dis = np.where(deg > 0, 1.0 / np.sqrt(np.maximum(deg, 1.0)), 0.0).astype(np.float32)
    wn = -dis[row] * dis[col]
    W = np.zeros((N, N), np.float32)
    np.add.at(W, (row, col), wn)

    # conv block matrices: L[(v,fi),(u,fo)] = Wt[fo,fi,0,dt]
    WtT = [np.ascontiguousarray(Wt[:, :, 0, d].T) for d in range(3)]  # (fi,fo)
    Z64 = np.zeros((F, F), np.float32)
    Lmid = np.block([[WtT[1], WtT[0]], [WtT[2], WtT[1]]]).astype(bf)
    Lprev = np.block([[Z64, Z64], [WtT[0], Z64]]).astype(bf)
    Lnext = np.block([[Z64, WtT[2]], [Z64, Z64]]).astype(bf)
    WrT = np.ascontiguousarray(Wr[:, :, 0, 0].T)
    WrP = np.block([[WrT, Z64], [Z64, WrT]]).astype(bf)
    WcP = np.stack([np.block([[W_cheb[k], Z64], [Z64, W_cheb[k]]]) for k in range(3)]
                   ).astype(bf)

    Wpk = np.stack([WcP[0], WcP[1], WcP[2], Lprev, Lmid, Lnext, WrP])

    Pf = np.zeros((P, PFW), np.float32)
    Pf[:, 0] = np.tile(gamma, 2)
    Pf[:, 1] = np.tile(-beta, 2)
    Pf[:, 2] = np.tile(b_cheb, 2)
    Pf[:, 3] = np.tile(bt + br, 2)
    Pf[:, 4:132] = np.eye(P, dtype=np.float32)

    VsTh = 0.5 * np.ascontiguousarray(Vs.T)
    vch = VsTh.sum(axis=0)                 # 0.5*colsum(Vs^T) sigmoid-fold row
    shared = {
        'bsh': (0.5 * bs[0]).astype(bf),
        'VsT': VsTh.astype(bf),
        'WT': np.ascontiguousarray(W.T).astype(bf),
        'Wpk': Wpk,
    }

    in_maps = []
    for core in range(8):
        b, h = core // 2, core % 2
        tmap = list(range(16)) if h == 0 else list(range(6, 16)) + list(range(6))
        Xp = X[b][:, :, tmap]                              # (N, F, 16)
        Xn = np.ascontiguousarray(Xp.transpose(0, 2, 1).reshape(N, T * F)).astype(bf)
        Xw = np.ascontiguousarray(Xp.transpose(2, 1, 0).reshape(8, P, N)).astype(bf)
        UW = np.zeros((8, P, 48), np.float32)
        for tp in range(16):
            s, v = tp // 2, tp % 2
            UW[s, 64 * v:64 * v + 64, tp] = Ws3
            UW[s, 64 * v:64 * v + 64, 32 + tp] = U3
        Pb = np.zeros((P, PBW), np.float32)
        Pb[:, 0:4] = U1.reshape(4, P).T
        Pb[:, 4:20] = np.vstack([Ws2, Ws2])
        VeTh = 0.5 * Ve[np.ix_(tmap, tmap)].T
        Pb[0:16, 20:36] = VeTh
        Pb[0:16, 36] = Ws1[tmap]
        Pb[0, 37:165] = 1.0
        Pb[:, 165:293] = np.eye(P, dtype=np.float32)
        Pb[0:64, 293:805] = U2
        Pb[0, 805:1061] = np.eye(T, dtype=np.float32).reshape(-1)
        # B2: (128, 2) block indicator * 1/64 for per-v mean over f
        Pb[0:64, 1061] = 1.0 / 64
        Pb[64:128, 1062] = 1.0 / 64
        # B2T: (2, 128) block indicator for broadcast back
        Pb[0, 1063:1127] = 1.0
        Pb[1, 1127:1191] = 1.0
        # sigmoid-fold rank-1 rows
        Pb[0, 1191:1207] = VeTh.sum(axis=0)
        Pb[0, 1207:1719] = vch
        Pfc = Pf.copy()
        Pfc[0:16, 132:148] = 0.5 * be[0][np.ix_(tmap, tmap)]
        m = dict(shared)
        m.update({
            'Xn': Xn, 'Xw': Xw, 'UW': UW.astype(bf),
            'Pb': Pb.astype(bf), 'Pf': Pfc,
        })
        in_maps.append(m)
    return in_maps


def kernel(**inputs):
    import sys
    if '/opt/trn_rl_repo' not in sys.path:
        sys.path.insert(0, '/opt/trn_rl_repo')
    from concourse.bass_utils import run_bass_kernel_spmd

    if 'nc' not in _CACHE:
        _CACHE['nc'] = _build_program()
    nc = _CACHE['nc']

    in_maps = _host_prep(inputs)
    res = run_bass_kernel_spmd(nc, in_maps, list(range(8)))
    out = np.zeros((B, N, F, T), np.float32)
    for core in range(8):
        b, h = core // 2, core % 2
        Z = np.asarray(res.results[core]['Zout']).astype(np.float32)
        # rows q*128 + v*64 + f, cols n  ->  (n, f, slot=2q+v)
        Zs = Z.reshape(NP, 2, F, N).transpose(3, 2, 0, 1).reshape(N, F, NSLOT)
        wstart = 0 if h == 0 else 6
        jlo = 0 if h == 0 else 2
        out[b, :, :, wstart + jlo:wstart + jlo + 8] = Zs[:, :, jlo:jlo + 8]
    return out


# revision 28
# speedup vs baseline: 1.5527x; 1.0564x over previous
"""ASTGCN block Trainium2 kernel (v2).

Strategy: 8 cores; core c handles batch b = c//2, time-half h = c%2 (8 output
timesteps each, data-parallel over B and T). Attention (temporal Et, spatial
S) is per-b and replicated on the 2 cores sharing a b. The sparse graph
propagation is reformulated as dense (N,N) matmuls: the edge-scatter of the
symmetric norm is accumulated host-side into a dense W (the +I/-I self-loop
terms cancel), so  prop1(h) = (W*S) @ h  and  prop2(h) = W @ h.

v2 changes vs baseline:
- Input DMAs ordered by first use (Pb/Pf/Xn first) and X tensors split in
  halves so attention matmuls start ~5us in instead of after all loads.
- Single activation-table regime: sigmoid via tanh (0.5*tanh(x/2)+0.5, in
  the exp table) and LN rstd via exp(-0.5*ln(var+eps)); only one table
  switch in the whole program (exp_and_others -> natural_log_exp...).
- LayerNorm runs in pair layout: per-pair stats via ones-block matmuls on
  PE (reduce over the f partition rows), rstd/-mu*rstd broadcast back with
  block matmuls; no transposes of the conv output at all.
- Output stored in pair layout as bf16; host does the final (f,n) -> (n,f)
  transpose and fp32 upcast.
- cheb -> conv -> LN -> store software-pipelined across the 5 timestep
  pairs to keep PE dense (p-state) and overlap store DMAs with compute.

Per-core time axis is PERMUTED so the program is identical SPMD: slot t' maps
to global t via tmap (identity for h=0, rotated by 6 for h=1); all
t-dependent weights (be, Ve, Ws1, UW) are permuted host-side to match.
"""

import numpy as np

B, N, F, T = 4, 512, 64, 16
P = 128
CH = N // P            # 4 n-chunks
NSLOT = 10             # cheb window timesteps per core (5 pairs)
NP = NSLOT // 2        # 5 pairs
LN_EPS = 1e-5

PBW = 1719             # packed bf16 constant width
PFW = 148              # packed f32 constant width

_CACHE = {}


def _build_program():
    import sys
    if '/opt/trn_rl_repo' not in sys.path:
        sys.path.insert(0, '/opt/trn_rl_repo')
    from contextlib import ExitStack
    import concourse.bass as bass
    import concourse.tile as tile
    from concourse import bacc, mybir

    dt = mybir.dt
    AL = mybir.AluOpType
    AF = mybir.ActivationFunctionType
    AX = mybir.AxisListType
    f32 = dt.float32
    bf16 = dt.bfloat16

    nc = bacc.Bacc("TRN2", target_bir_lowering=False, debug=False, num_devices=1)

    def din(name, shape, d=bf16):
        return nc.dram_tensor(name, list(shape), d, kind="ExternalInput").ap()

    XnD   = din("Xn", (N, T * F))
    XwD   = din("Xw", (8, P, N))
    UWD   = din("UW", (8, P, 48))
    bshD  = din("bsh", (N, N))          # 0.5 * bs
    VsTD  = din("VsT", (N, N))
    WTD   = din("WT", (N, N))
    WpkD  = din("Wpk", (7, P, P))
    PbD   = din("Pb", (P, PBW))
    PfD   = din("Pf", (P, PFW), f32)
    ZoutD = nc.dram_tensor("Zout", [NP * P, N], bf16, kind="ExternalOutput").ap()

    with tile.TileContext(nc) as tc, ExitStack() as ctx:
        sg = ctx.enter_context(tc.tile_pool(name="sg", bufs=1))
        big = ctx.enter_context(tc.tile_pool(name="big", bufs=5, space="PSUM"))
        sml = ctx.enter_context(tc.tile_pool(name="sml", bufs=2, space="PSUM"))
        hlf = ctx.enter_context(tc.tile_pool(name="hlf", bufs=1, space="PSUM"))
        xhp = ctx.enter_context(tc.tile_pool(name="xhp", bufs=7))
        txp = ctx.enter_context(tc.tile_pool(name="txp", bufs=5))
        lnp = ctx.enter_context(tc.tile_pool(name="lnp", bufs=5))

        # ------------- input DMAs, ordered by first use -------------
        Pb = sg.tile([P, PBW], bf16, tag="pb")
        nc.sync.dma_start(out=Pb[:], in_=PbD)
        Pf = sg.tile([P, PFW], f32, tag="pf")
        nc.sync.dma_start(out=Pf[:], in_=PfD)
        XnA = sg.tile([P, 2, T * F], bf16, tag="xna")
        XnB = sg.tile([P, 2, T * F], bf16, tag="xnb")
        XnDr = XnD.rearrange("(k p) t -> p k t", k=CH)
        nc.sync.dma_start(out=XnA[:], in_=XnDr[:, 0:2, :])
        nc.sync.dma_start(out=XnB[:], in_=XnDr[:, 2:4, :])
        UWAll = sg.tile([P, 8, 48], bf16, tag="uwall")
        nc.sync.dma_start(out=UWAll[:], in_=UWD.rearrange("s p n -> p s n"))
        XwA = sg.tile([P, 4, N], bf16, tag="xwa")
        XwB = sg.tile([P, 4, N], bf16, tag="xwb")
        XwDr = XwD.rearrange("s p n -> p s n")
        nc.sync.dma_start(out=XwA[:], in_=XwDr[:, 0:4, :])
        nc.sync.dma_start(out=XwB[:], in_=XwDr[:, 4:8, :])
        bsAll = sg.tile([P, CH, N], bf16, tag="bsall")
        nc.sync.dma_start(out=bsAll[:], in_=bshD.rearrange("(k p) n -> p k n", k=CH))
        VsTAll = sg.tile([P, CH, N], bf16, tag="vstall")
        nc.sync.dma_start(out=VsTAll[:], in_=VsTD.rearrange("(k p) n -> p k n", k=CH))
        WTAll = sg.tile([P, CH, N], bf16, tag="wtall")
        nc.sync.dma_start(out=WTAll[:], in_=WTD.rearrange("(k p) n -> p k n", k=CH))
        Wpk = sg.tile([P, 7, P], bf16, tag="wpk")
        nc.sync.dma_start(out=Wpk[:], in_=WpkD.rearrange("w p c -> p w c"))

        Xn = [XnA[:, 0, :], XnA[:, 1, :], XnB[:, 0, :], XnB[:, 1, :]]
        Xw = [XwA[:, s, :] for s in range(4)] + [XwB[:, s, :] for s in range(4)]
        UW = [UWAll[:, s, :] for s in range(8)]
        bsh = [bsAll[:, k, :] for k in range(CH)]
        VsT = [VsTAll[:, k, :] for k in range(CH)]
        WT = [WTAll[:, k, :] for k in range(CH)]
        WcP = [Wpk[:, k, :] for k in range(3)]
        Lprev, Lmid, Lnext, WrP = (Wpk[:, 3, :], Wpk[:, 4, :], Wpk[:, 5, :],
                                   Wpk[:, 6, :])
        # packed bf16 layout
        U1r = Pb[:, 0:4]
        Ws2d = Pb[:, 4:20]
        VeT = Pb[0:16, 20:36]
        Ws1 = Pb[0:16, 36:37]
        ones1 = Pb[0:1, 37:165]
        I128b = Pb[:, 165:293]
        U2 = Pb[0:64, 293:805]
        I16r = Pb[0:1, 805:1061]     # I16 rows flattened: e_t = [0:1, 16t:16t+16]
        B2 = Pb[:, 1061:1063]        # (128,2) block col-indicator * 1/64
        B2T = Pb[0:2, 1063:1191]     # (2,128) block row-indicator
        hcVe = Pb[0:1, 1191:1207]    # 0.5*colsum(VeT')  [sigmoid-fold row]
        vch = Pb[0:1, 1207:1719]     # 0.5*colsum(VsT')  [sigmoid-fold row]
        # packed f32 layout
        gamP = Pf[:, 0:1]
        nbetP = Pf[:, 1:2]           # -beta
        bch = Pf[:, 2:3]
        btr = Pf[:, 3:4]
        I128f = Pf[:, 4:132]
        bePh = Pf[0:16, 132:148]     # 0.5 * be (permuted)

        zerot = sg.tile([P, N], bf16, tag="zerot")
        nc.vector.memset(zerot[:], 0.0)
        epsP = sg.tile([P, 1], f32, tag="epsP")
        nc.vector.memset(epsP[:], LN_EPS)

        # persistent sbuf intermediates
        G = [sg.tile([P, N], bf16, tag=f"g{k}", name=f"g{k}") for k in range(CH)]
        Ex = [sg.tile([P, N], bf16, tag=f"ex{k}", name=f"ex{k}") for k in range(CH)]
        A1T = [sg.tile([P, N], bf16, tag=f"a1t{k}", name=f"a1t{k}") for k in range(CH)]
        dSv = [sg.tile([P, 1], f32, tag=f"dsv{k}", name=f"dsv{k}") for k in range(CH)]
        Tx0n = [sg.tile([P, T * F], bf16, tag=f"tx0n{k}", name=f"tx0n{k}")
                for k in range(CH)]
        dSB = sg.tile([P, N], bf16, tag="dsb")

        # =====================================================
        # Attention phase
        # =====================================================
        # ---- lhs0[(t,f)] = sum_n U1[n] X[n,(t,f)]  -> (1,1024)
        L0a = sml.tile([1, 512], f32, tag="sml", name="l0a")
        for k in range(CH):
            nc.tensor.matmul(L0a[:], U1r[:, k:k + 1], Xn[k][:, 0:512],
                             start=(k == 0), stop=(k == CH - 1))
        L0b = sml.tile([1, 512], f32, tag="sml", name="l0b")
        for k in range(CH):
            nc.tensor.matmul(L0b[:], U1r[:, k:k + 1], Xn[k][:, 512:1024],
                             start=(k == 0), stop=(k == CH - 1))
        lhs0row = sg.tile([1, T * F], bf16, tag="lhs0row")
        nc.vector.tensor_copy(lhs0row[:, 0:512], L0a[:])
        nc.vector.tensor_copy(lhs0row[:, 512:1024], L0b[:])
        # reshape to (64,16) via 16 rank-1 matmuls against identity rows
        l0Fp = sml.tile([F, T], f32, tag="sml", name="l0fp")
        for t in range(T):
            nc.tensor.matmul(l0Fp[:], lhs0row[0:1, 64 * t:64 * t + 64],
                             I16r[0:1, 16 * t:16 * t + 16],
                             start=(t == 0), stop=(t == T - 1))
        lhs0F = sg.tile([F, T], bf16, tag="lhs0f")
        nc.vector.tensor_copy(lhs0F[:], l0Fp[:])

        # ---- lhs2T chunks (n,16) = U2[:,chunk].T @ lhs0F
        lhs2T = []
        for k in range(CH):
            pt = sml.tile([P, T], f32, tag="sml", name="l2t")
            nc.tensor.matmul(pt[:], U2[:, k * P:(k + 1) * P], lhs0F[:],
                             start=True, stop=True)
            st = sg.tile([P, T], bf16, tag=f"l2ts{k}", name=f"l2ts{k}")
            nc.vector.tensor_copy(st[:], pt[:])
            lhs2T.append(st)

        # ---- R48: rows 0:16 rhs3T (Ws3), rows 32:48 rhs_tT (U3)
        R48p = sml.tile([48, N], f32, tag="sml", name="r48")
        for s in range(8):
            nc.tensor.matmul(R48p[:], UW[s][:, :], Xw[s][:, :],
                             start=(s == 0), stop=(s == 7))
        R48 = sg.tile([48, N], bf16, tag="r48s")
        nc.scalar.copy(R48[:], R48p[:])

        # ---- rhs_tn chunks: transpose R48[32:48]
        rhs_tn = []
        for k in range(CH):
            pt = hlf.tile([P, T], bf16, tag="hlf", name="rtn")
            nc.tensor.transpose(pt[:], R48[32:48, k * P:(k + 1) * P],
                                I128b[32:48, 32:48])
            st = sg.tile([P, T], bf16, tag=f"rtns{k}", name=f"rtns{k}")
            nc.vector.tensor_copy(st[:], pt[:])
            rhs_tn.append(st)

        # ---- P0 (16,16) = lhs_t @ rhs_t ; sigmoid via tanh:
        # sig = 0.5*tanh(0.5*(P0+be)) + 0.5
        P0p = sml.tile([T, T], f32, tag="sml", name="p0")
        for k in range(CH):
            nc.tensor.matmul(P0p[:], lhs2T[k][:], rhs_tn[k][:],
                             start=(k == 0), stop=(k == CH - 1))
        # sigmoid fold: sig = 0.5*tanh(0.5*(P0+be)) + 0.5; the affine is folded
        # into the E1T matmul (VeT is 0.5-scaled host-side, hcVe rank-1 term)
        sig = sg.tile([T, T], bf16, tag="sig")
        nc.vector.scalar_tensor_tensor(sig[:], P0p[:], 0.5, bePh[:],
                                       op0=AL.mult, op1=AL.add)
        nc.scalar.activation(sig[:], sig[:], AF.Tanh)

        # ---- E1^T = sigmoid^T @ Ve^T ; softmax over free dim
        E1Tp = sml.tile([T, T], f32, tag="sml", name="e1t")
        nc.tensor.matmul(E1Tp[:], sig[:], VeT[:], start=True, stop=False)
        nc.tensor.matmul(E1Tp[:], ones1[0:1, 0:16], hcVe[:],
                         start=False, stop=True)
        E1Ts = sg.tile([T, T], bf16, tag="e1ts")
        nc.vector.tensor_copy(E1Ts[:], E1Tp[:])
        # values are O(1e-1): skip the max-subtraction for softmax
        sume = sg.tile([T, 1], f32, tag="sume")
        EtT = sg.tile([T, T], bf16, tag="ett")
        nc.scalar.activation(EtT[:], E1Ts[:], AF.Exp,
                             scale=1.0, accum_out=sume[:, 0:1])
        rse = sg.tile([T, 1], f32, tag="rse")
        nc.vector.reciprocal(rse[:], sume[:])
        nc.vector.tensor_scalar(EtT[:], EtT[:], rse[:, 0:1], None, op0=AL.mult)
        Etp = hlf.tile([T, T], bf16, tag="hlf", name="etp")
        nc.tensor.transpose(Etp[:], EtT[:], I128b[0:16, 0:16])
        Et = sg.tile([T, T], bf16, tag="et")
        nc.vector.tensor_copy(Et[:], Etp[:])

        # ---- w1e row (1,16) = Ws1.T @ EtT ; broadcast to (128,16)
        w1p = sml.tile([1, T], f32, tag="sml", name="w1p")
        nc.tensor.matmul(w1p[:], Ws1[:], EtT[:], start=True, stop=True)
        w1row = sg.tile([1, T], bf16, tag="w1row")
        nc.scalar.copy(w1row[:], w1p[:])
        w1Bp = sml.tile([P, T], f32, tag="sml", name="w1bp")
        nc.tensor.matmul(w1Bp[:], ones1[:], w1row[:], start=True, stop=True)
        w1B = sg.tile([P, T], bf16, tag="w1b")
        nc.vector.tensor_copy(w1B[:], w1Bp[:])

        # ---- w1Bpair[p=(v,f), s] = w1e[2s+v]
        w1Bp2 = sg.tile([P, 8], bf16, tag="w1bp2")
        nc.vector.tensor_copy(w1Bp2[0:64, :], w1B[0:64, 0:T:2])
        nc.vector.tensor_copy(w1Bp2[64:128, :], w1B[64:128, 1:T:2])
        # ---- Ws2wP[p=(v,f), s, t] = Ws2d[p,t] * w1e[2s+v]   (128, 8, 16)
        Ws2w = sg.tile([P, 8, T], bf16, tag="ws2w")
        nc.vector.tensor_tensor(
            Ws2w[:],
            Ws2d[:].unsqueeze(1).broadcast_to((P, 8, T)),
            w1Bp2[:].unsqueeze(2).broadcast_to((P, 8, T)),
            op=AL.mult)

        # ---- lhs_sT (16, 512) = sum_t1 (Ws2*w1e[t1]).T @ X^T[t1]
        lsTp = sml.tile([T, N], f32, tag="sml", name="lst")
        for s in range(8):
            nc.tensor.matmul(lsTp[:], Ws2w[:, s, :], Xw[s][:, :],
                             start=(s == 0), stop=(s == 7))
        lsT = sg.tile([T, N], bf16, tag="lsts")
        nc.scalar.copy(lsT[:], lsTp[:])

        # ---- rhs_s (16, 512) = Et-weighted rhs3
        rsp = sml.tile([T, N], f32, tag="sml", name="rsp")
        nc.tensor.matmul(rsp[:], Et[:], R48[0:16, :], start=True, stop=True)
        rss = sg.tile([T, N], bf16, tag="rss")
        nc.scalar.copy(rss[:], rsp[:])

        # ---- P chunks; G holds tanh(0.5*P + bsh); the sigmoid affine is
        # folded into M1T (VsT 0.5-scaled host-side + vch rank-1 term)
        for k in range(CH):
            Pp = big.tile([P, N], f32, tag="big", name="pp")
            nc.tensor.matmul(Pp[:], lsT[:, k * P:(k + 1) * P], rss[:],
                             start=True, stop=True)
            nc.vector.scalar_tensor_tensor(G[k][:], Pp[:], 0.5, bsh[k][:],
                                           op0=AL.mult, op1=AL.add)
            nc.scalar.activation(G[k][:], G[k][:], AF.Tanh)

        # ---- M1T chunks (c-part, r) + masked softmax -> A1T, dS
        for c in range(CH):
            Mp = big.tile([P, N], f32, tag="big", name="mp")
            for k in range(CH):
                nc.tensor.matmul(Mp[:], G[k][:, c * P:(c + 1) * P], VsT[k][:],
                                 start=(k == 0), stop=False)
            nc.tensor.matmul(Mp[:], ones1[0:1, 0:128], vch[:],
                             start=False, stop=True)
            sme = sg.tile([P, 1], f32, tag=f"sme{c}", name=f"sme{c}")
            nc.scalar.activation(Ex[c][:], Mp[:], AF.Exp,
                                 scale=1.0, accum_out=sme[:, 0:1])
            rcp = sg.tile([P, 1], f32, tag=f"rcp{c}", name=f"rcp{c}")
            nc.vector.reciprocal(rcp[:], sme[:])
            # A1T = (Ex * rcp) * WT   (= S^T o W^T)
            nc.vector.scalar_tensor_tensor(A1T[c][:], Ex[c][:], rcp[:, 0:1],
                                           WT[c][:], op0=AL.mult, op1=AL.mult)
            # diag: dS = sum_r (Ex*rcp)*I over the diagonal block
            dtmp = sg.tile([P, P], bf16, tag="dtmp")
            nc.vector.scalar_tensor_tensor(dtmp[:], Ex[c][:, c * P:(c + 1) * P],
                                           rcp[:, 0:1], I128b[:],
                                           op0=AL.mult, op1=AL.mult)
            nc.vector.tensor_reduce(dSv[c][:], dtmp[:], axis=AX.X, op=AL.add)

        # ---- dS row + broadcast tile (128, 512)
        dSrp = sml.tile([1, N], f32, tag="sml", name="dsrp")
        for c in range(CH):
            nc.tensor.transpose(dSrp[:, c * P:(c + 1) * P], dSv[c][:], I128f[:])
        dSrow = sg.tile([1, N], bf16, tag="dsrow")
        nc.scalar.copy(dSrow[:], dSrp[:])
        dSBp = sml.tile([P, N], f32, tag="sml", name="dsbp")
        nc.tensor.matmul(dSBp[:], ones1[:], dSrow[:], start=True, stop=True)
        nc.scalar.copy(dSB[:], dSBp[:])

        # ---- Tx0 in n-layout (all t at once)
        for k in range(CH):
            nc.vector.tensor_scalar(Tx0n[k][:], Xn[k][:], dSv[k][:, 0:1], None,
                                    op0=AL.mult)

        # =====================================================
        # Cheb + conv + LN, software-pipelined per pair
        # =====================================================
        Tx0P = {}
        TAp = {}
        Tx1T = {}
        ptA = {}
        Tx1n = {}
        TBp = {}
        Tx2T = {}
        TCp = {}
        XhP = {-1: zerot, NP: zerot}
        TDp = {}
        ZT = {}

        def e_tx0p(q):
            t = txp.tile([P, N], bf16, tag="tx0p", name=f"tx0p{q}")
            nc.gpsimd.tensor_tensor(t[:], Xw[q][:], dSB[:], op=AL.mult)
            Tx0P[q] = t

        def e_ta(q):
            p = big.tile([P, N], f32, tag="big", name=f"ta{q}")
            for k in range(CH):
                lhs = Tx0n[k][:, 2 * q * F:(2 * q + 2) * F]
                nc.tensor.matmul(p[:], lhs, A1T[k][:],
                                 start=(k == 0), stop=(k == CH - 1))
            TAp[q] = p

        def e_b(q):
            t = txp.tile([P, N], bf16, tag="tx1t", name=f"tx1t{q}")
            nc.vector.tensor_copy(t[:], TAp[q][:])
            Tx1T[q] = t

        def e_c(q):
            p = hlf.tile([P, N], bf16, tag="hlf", name=f"pta{q}")
            for k in range(CH):
                nc.tensor.transpose(p[:, k * P:(k + 1) * P],
                                    Tx1T[q][:, k * P:(k + 1) * P], I128b[:])
            ptA[q] = p

        def e_d(q):
            t = txp.tile([P, N], bf16, tag="tx1n", name=f"tx1n{q}")
            nc.scalar.copy(t[:], ptA[q][:])
            Tx1n[q] = t

        def e_e(q):
            p = big.tile([P, N], f32, tag="big", name=f"tb{q}")
            for k in range(CH):
                nc.tensor.matmul(p[:], Tx1n[q][:, k * P:(k + 1) * P], WT[k][:],
                                 start=(k == 0), stop=(k == CH - 1))
            TBp[q] = p

        def e_f(q):
            t = txp.tile([P, N], bf16, tag="tx2t", name=f"tx2t{q}")
            nc.vector.scalar_tensor_tensor(t[:], TBp[q][:], 2.0, Tx0P[q][:],
                                           op0=AL.mult, op1=AL.subtract)
            Tx2T[q] = t

        def e_g(q):
            p = big.tile([P, N], f32, tag="big", name=f"tc{q}")
            nc.tensor.matmul(p[:], WcP[0][:], Tx0P[q][:], start=True, stop=False)
            nc.tensor.matmul(p[:], WcP[1][:], Tx1T[q][:], start=False, stop=False)
            nc.tensor.matmul(p[:], WcP[2][:], Tx2T[q][:], start=False, stop=True)
            TCp[q] = p

        def e_h(q):
            t = xhp.tile([P, N], bf16, tag="xh", name=f"xh{q}")
            nc.scalar.activation(t[:], TCp[q][:], AF.Relu, bias=bch[:, 0:1],
                                 scale=1.0)
            XhP[q] = t

        def e_i(q):
            p = big.tile([P, N], f32, tag="big", name=f"td{q}")
            nc.tensor.matmul(p[:], Lprev[:], XhP[q - 1][:], start=True, stop=False)
            nc.tensor.matmul(p[:], Lmid[:], XhP[q][:], start=False, stop=False)
            nc.tensor.matmul(p[:], Lnext[:], XhP[q + 1][:], start=False, stop=False)
            nc.tensor.matmul(p[:], WrP[:], Xw[q][:], start=False, stop=True)
            TDp[q] = p

        def e_j(q):
            t = lnp.tile([P, N], bf16, tag="zt", name=f"zt{q}")
            nc.scalar.activation(t[:], TDp[q][:], AF.Relu, bias=btr[:, 0:1],
                                 scale=1.0)
            ZT[q] = t

        RSTD = {}
        NMR = {}
        SQ = {}

        def e_sq(q):
            t = lnp.tile([P, N], bf16, tag="sq", name=f"sq{q}")
            nc.gpsimd.tensor_tensor(t[:], ZT[q][:], ZT[q][:], op=AL.mult)
            SQ[q] = t

        def e_ln1(q):
            z = ZT[q]
            sq = SQ[q]
            # stats: rows 0:2 mean per v, rows 32:34 E[x^2] per v (B2 = 1/64)
            # (matmul out base partition must be 0/32/64)
            s12 = sml.tile([34, N], f32, tag="sml", name=f"s12{q}")
            nc.tensor.matmul(s12[0:2, :], B2[:], z[:], start=True, stop=True)
            nc.tensor.matmul(s12[32:34, :], B2[:], sq[:], start=True, stop=True)
            mu = lnp.tile([2, N], f32, tag="mu", name=f"mu{q}")
            nc.scalar.copy(mu[:], s12[0:2, :])
            mu2 = lnp.tile([2, N], f32, tag="mu2", name=f"mu2{q}")
            nc.vector.tensor_tensor(mu2[:], mu[:], mu[:], op=AL.mult)
            # var+eps = (msq + eps) - mu^2 in one stt
            var = lnp.tile([2, N], f32, tag="var", name=f"var{q}")
            nc.vector.scalar_tensor_tensor(var[:], s12[32:34, :], LN_EPS, mu2[:],
                                           op0=AL.add, op1=AL.subtract)
            # rstd = sqrt(1/(var+eps)); approx recip is ~18 bits, plenty
            rv = lnp.tile([2, N], f32, tag="rv", name=f"rv{q}")
            nc.vector.reciprocal_approx_fast(rv[:], var[:])
            rstd = lnp.tile([2, N], bf16, tag="rstd", name=f"rstd{q}")
            nc.scalar.activation(rstd[:], rv[:], AF.Sqrt)
            RSTD[q] = rstd
            # nmr = mu * rstd
            nmr = lnp.tile([2, N], bf16, tag="nmr", name=f"nmr{q}")
            nc.gpsimd.tensor_tensor(nmr[:], mu[:], rstd[:], op=AL.mult)
            NMR[q] = nmr

        def e_ln2(q):
            z = ZT[q]
            # broadcast to 128 partitions
            rBp = big.tile([P, N], f32, tag="big", name=f"rbp{q}")
            nc.tensor.matmul(rBp[:], B2T[:], RSTD[q][:], start=True, stop=True)
            nBp = big.tile([P, N], f32, tag="big", name=f"nbp{q}")
            nc.tensor.matmul(nBp[:], B2T[:], NMR[q][:], start=True, stop=True)
            # w = (z*rB)*gam - (nB*gam - bet)
            u = lnp.tile([P, N], bf16, tag="u", name=f"u{q}")
            nc.vector.tensor_tensor(u[:], z[:], rBp[:], op=AL.mult)
            nB2 = lnp.tile([P, N], bf16, tag="nb2", name=f"nb2{q}")
            nc.scalar.activation(nB2[:], nBp[:], AF.Identity,
                                 bias=nbetP[:, 0:1], scale=gamP[:, 0:1])
            w = lnp.tile([P, N], bf16, tag="w", name=f"w{q}")
            nc.vector.scalar_tensor_tensor(w[:], u[:], gamP[:, 0:1], nB2[:],
                                           op0=AL.mult, op1=AL.subtract)
            nc.sync.dma_start(out=ZoutD[q * P:(q + 1) * P, :], in_=w[:])

        # pipeline drive, depth 7: every PE group's inputs are produced in a
        # PREVIOUS iteration, so the in-order PE queue never head-of-line
        # blocks. Stage distances: TA@0, trans@1, TB@2, TC@3, TD@4, ln1@5,
        # ln2@7. Cross-engine hops (stt, relu, copies) happen within the
        # iteration that produced their psum input.
        def live(q):
            return 0 <= q < NP

        for i in range(NP + 7):
            if live(i - 7):
                e_ln2(i - 7)
            if live(i - 1):
                e_c(i - 1)
            if live(i):
                if i == 0:
                    e_tx0p(0)
                e_ta(i)
            if live(i - 1):
                e_d(i - 1)
            if live(i - 2):
                e_e(i - 2)
                e_f(i - 2)
            if live(i - 3):
                e_g(i - 3)
                e_h(i - 3)
            if live(i):
                e_b(i)
                if live(i + 1):
                    e_tx0p(i + 1)
            if live(i - 4):
                e_i(i - 4)
                e_j(i - 4)
                e_sq(i - 4)
            if live(i - 5):
                e_ln1(i - 5)

    nc.compile()
    return nc


def _host_prep(inputs):
    import ml_dtypes
    bf = ml_dtypes.bfloat16

    X = np.asarray(inputs['X'], np.float32)
    edge_index = np.asarray(inputs['edge_index'])
    U1 = np.asarray(inputs['U1'], np.float32)
    U2 = np.asarray(inputs['U2'], np.float32)
    U3 = np.asarray(inputs['U3'], np.float32)
    be = np.asarray(inputs['be'], np.float32)
    Ve = np.asarray(inputs['Ve'], np.float32)
    Ws1 = np.asarray(inputs['Ws1'], np.float32)
    Ws2 = np.asarray(inputs['Ws2'], np.float32)
    Ws3 = np.asarray(inputs['Ws3'], np.float32)
    bs = np.asarray(inputs['bs'], np.float32)
    Vs = np.asarray(inputs['Vs'], np.float32)
    W_cheb = np.asarray(inputs['W_cheb'], np.float32)
    b_cheb = np.asarray(inputs['b_cheb'], np.float32)
    Wt = np.asarray(inputs['Wt'], np.float32)
    bt = np.asarray(inputs['bt'], np.float32)
    Wr = np.asarray(inputs['Wr'], np.float32)
    br = np.asarray(inputs['br'], np.float32)
    gamma = np.asarray(inputs['gamma'], np.float32)
    beta = np.asarray(inputs['beta'], np.float32)

    # dense symmetric-norm matrix (self-loop +I/-I terms cancel)
    row, col = edge_index[0].astype(np.int64), edge_index[1].astype(np.int64)
    deg = np.zeros(N, np.float32)
    np.add.at(deg, row, 1.0)
    dis = np.where(deg > 0, 1.0 / np.sqrt(np.maximum(deg, 1.0)), 0.0).astype(np.float32)
    wn = -dis[row] * dis[col]
    W = np.zeros((N, N), np.float32)
    np.add.at(W, (row, col), wn)

    # conv block matrices: L[(v,fi),(u,fo)] = Wt[fo,fi,0,dt]
    WtT = [np.ascontiguousarray(Wt[:, :, 0, d].T) for d in range(3)]  # (fi,fo)
    Z64 = np.zeros((F, F), np.float32)
    Lmid = np.block([[WtT[1], WtT[0]], [WtT[2], WtT[1]]]).astype(bf)
    Lprev = np.block([[Z64, Z64], [WtT[0], Z64]]).astype(bf)
    Lnext = np.block([[Z64, WtT[2]], [Z64, Z64]]).astype(bf)
    WrT = np.ascontiguousarray(Wr[:, :, 0, 0].T)
    WrP = np.block([[WrT, Z64], [Z64, WrT]]).astype(bf)
    WcP = np.stack([np.block([[W_cheb[k], Z64], [Z64, W_cheb[k]]]) for k in range(3)]
                   ).astype(bf)

    Wpk = np.stack([WcP[0], WcP[1], WcP[2], Lprev, Lmid, Lnext, WrP])

    Pf = np.zeros((P, PFW), np.float32)
    Pf[:, 0] = np.tile(gamma, 2)
    Pf[:, 1] = np.tile(-beta, 2)
    Pf[:, 2] = np.tile(b_cheb, 2)
    Pf[:, 3] = np.tile(bt + br, 2)
    Pf[:, 4:132] = np.eye(P, dtype=np.float32)

    VsTh = 0.5 * np.ascontiguousarray(Vs.T)
    vch = VsTh.sum(axis=0)                 # 0.5*colsum(Vs^T) sigmoid-fold row
    shared = {
        'bsh': (0.5 * bs[0]).astype(bf),
        'VsT': VsTh.astype(bf),
        'WT': np.ascontiguousarray(W.T).astype(bf),
        'Wpk': Wpk,
    }

    in_maps = []
    for core in range(8):
        b, h = core // 2, core % 2
        tmap = list(range(16)) if h == 0 else list(range(6, 16)) + list(range(6))
        Xp = X[b][:, :, tmap]                              # (N, F, 16)
        Xn = np.ascontiguousarray(Xp.transpose(0, 2, 1).reshape(N, T * F)).astype(bf)
        Xw = np.ascontiguousarray(Xp.transpose(2, 1, 0).reshape(8, P, N)).astype(bf)
        UW = np.zeros((8, P, 48), np.float32)
        for tp in range(16):
            s, v = tp // 2, tp % 2
            UW[s, 64 * v:64 * v + 64, tp] = Ws3
            UW[s, 64 * v:64 * v + 64, 32 + tp] = U3
        Pb = np.zeros((P, PBW), np.float32)
        Pb[:, 0:4] = U1.reshape(4, P).T
        Pb[:, 4:20] = np.vstack([Ws2, Ws2])
        VeTh = 0.5 * Ve[np.ix_(tmap, tmap)].T
        Pb[0:16, 20:36] = VeTh
        Pb[0:16, 36] = Ws1[tmap]
        Pb[0, 37:165] = 1.0
        Pb[:, 165:293] = np.eye(P, dtype=np.float32)
        Pb[0:64, 293:805] = U2
        Pb[0, 805:1061] = np.eye(T, dtype=np.float32).reshape(-1)
        # B2: (128, 2) block indicator * 1/64 for per-v mean over f
        Pb[0:64, 1061] = 1.0 / 64
        Pb[64:128, 1062] = 1.0 / 64
        # B2T: (2, 128) block indicator for broadcast back
        Pb[0, 1063:1127] = 1.0
        Pb[1, 1127:1191] = 1.0
        # sigmoid-fold rank-1 rows
        Pb[0, 1191:1207] = VeTh.sum(axis=0)
        Pb[0, 1207:1719] = vch
        Pfc = Pf.copy()
        Pfc[0:16, 132:148] = 0.5 * be[0][np.ix_(tmap, tmap)]
        m = dict(shared)
        m.update({
            'Xn': Xn, 'Xw': Xw, 'UW': UW.astype(bf),
            'Pb': Pb.astype(bf), 'Pf': Pfc,
        })
        in_maps.append(m)
    return in_maps


def kernel(**inputs):
    import sys
    if '/opt/trn_rl_repo' not in sys.path:
        sys.path.insert(0, '/opt/trn_rl_repo')
    from concourse.bass_utils import run_bass_kernel_spmd

    if 'nc' not in _CACHE:
        _CACHE['nc'] = _build_program()
    nc = _CACHE['nc']

    in_maps = _host_prep(inputs)
    res = run_bass_kernel_spmd(nc, in_maps, list(range(8)))
    out = np.zeros((B, N, F, T), np.float32)
    for core in range(8):
        b, h = core // 2, core % 2
        Z = np.asarray(res.results[core]['Zout']).astype(np.float32)
        # rows q*128 + v*64 + f, cols n  ->  (n, f, slot=2q+v)
        Zs = Z.reshape(NP, 2, F, N).transpose(3, 2, 0, 1).reshape(N, F, NSLOT)
        wstart = 0 if h == 0 else 6
        jlo = 0 if h == 0 else 2
        out[b, :, :, wstart + jlo:wstart + jlo + 8] = Zs[:, :, jlo:jlo + 8]
    return out
